# revision 17
# baseline (speedup 1.0000x reference)
"""C3D-style circulant-block 3D CNN forward pass on 8 Trainium2 NeuronCores.

Sharding: data-parallel over batch (8 samples -> 8 cores). Training-mode
BatchNorm batch statistics are combined across cores with small per-chunk
f32 AllReduces of (mean, E[x^2]) per channel, issued as soon as each
output-channel chunk (or spatial piece, for single-chunk layers) finishes
so the collective overlaps the remaining matmuls.

Device kernel per core (per sample):
  conv1 via host-side im2col (K=81) -> col-tiled matmuls that compute two
  z-planes concurrently (output channels 0-63 on partitions 0-63 for even
  z, 64-127 for odd z). PSUM->bf16 on ACT, batched bn_stats on DVE, both
  maxpool stages on GpSimd.
  conv2: input channels x kd{0,1} packed into K=128 (B2) + kd=2 handled by
  a row-shift-duplicated plane (C2: parts 64-127 hold the plane shifted by
  one row) so kh taps pair up -> 15 matmuls per tile instead of 18.
  conv3a..5b as shift-and-accumulate implicit GEMM over per-chunk padded
  SBUF inputs; per chunk: stats -> AllReduce -> BN prep -> pool -> fused
  BN+ReLU apply into the next layer's padded chunk, all overlapped with
  the next chunk's matmuls.
  Tail: per-chunk special pool5 -> global mean (folded /16 into FC
  weights) -> FC matmul -> logits.
"""

import numpy as np
import ml_dtypes

import concourse.bass as bass
import concourse.mybir as mybir
import concourse.tile as tile
from concourse import bacc
from concourse.bass_utils import run_bass_kernel_spmd

F32 = mybir.dt.float32
BF16 = mybir.dt.bfloat16
NPBF16 = ml_dtypes.bfloat16
RELU = mybir.ActivationFunctionType.Relu
COPY = mybir.ActivationFunctionType.Copy
SQRT = mybir.ActivationFunctionType.Sqrt
IDENT = mybir.ActivationFunctionType.Identity
ADD = mybir.AluOpType.add
EPS = 1e-5
N_CORES = 8

# name, Cin, Cout, D, H, W, R(rows/tile), zpair, pooled
GEN_LAYERS = [
    ("3a", 128, 256, 8, 28, 28, 14, False, False),
    ("3b", 256, 256, 8, 28, 28, 14, False, True),
    ("4a", 256, 512, 4, 14, 14, 14, True, False),
    ("4b", 512, 512, 4, 14, 14, 14, True, True),
    ("5a", 512, 512, 2, 7, 7, 7, True, False),
    ("5b", 512, 512, 2, 7, 7, 7, True, None),  # None -> raw stage (special pool)
]

TAPS = [(kd, kh, kw) for kd in range(3) for kh in range(3) for kw in range(3)]


def circ_expand_np(c):
    c = np.asarray(c, np.float32)
    P, Q, b = c.shape[0], c.shape[1], c.shape[2]
    r = np.arange(b)
    idx = (r[:, None] - r[None, :]) % b
    w = c[:, :, idx]  # (P, Q, b, b, k, k, k)
    w = np.transpose(w, (0, 2, 1, 3, 4, 5, 6))
    return w.reshape(P * b, Q * b, *c.shape[3:])


def pack_w_generic(wd, Kch, Mch):
    # wd (Co, Ci, 3,3,3) -> [Mch, 128ci, Kch, 27, 128co] bf16
    Co, Ci = wd.shape[0], wd.shape[1]
    wt = wd.transpose(1, 2, 3, 4, 0)  # (Ci, kd,kh,kw, Co)
    wt = wt.reshape(Kch, 128, 27, Mch, 128)
    wt = wt.transpose(3, 1, 0, 2, 4)  # (m, ci, c, t, co)
    return np.ascontiguousarray(wt, dtype=NPBF16)


def host_prep(inputs):
    g = {k: np.asarray(v, np.float32) for k, v in inputs.items()}
    shared = {}
    # conv1
    w1 = g["conv1_w"]  # (64, 3, 3,3,3)
    shared["w1"] = np.ascontiguousarray(
        w1.transpose(1, 2, 3, 4, 0).reshape(81, 64), dtype=NPBF16)
    # conv2 (kd-packed + kd2 row-shift-paired)
    w2 = circ_expand_np(g["c2"])  # (128, 64, 3,3,3)
    w2t = w2.transpose(2, 1, 3, 4, 0)  # (kd, ci, kh, kw, co)
    shared["w2a"] = np.ascontiguousarray(
        w2t[0:2].reshape(128, 9, 128), dtype=NPBF16)
    # kd=2: pair (kh0,kh1) via row-shifted upper partitions; kh2 alone with
    # zeroed upper-partition weights.
    w2b_pair = np.zeros((128, 3, 128), np.float32)
    w2b_pair[0:64] = w2t[2, :, 0]      # (ci, kw, co) for kh=0
    w2b_pair[64:128] = w2t[2, :, 1]    # kh=1 (upper = plane shifted +1 row)
    shared["w2bp"] = np.ascontiguousarray(w2b_pair, dtype=NPBF16)
    w2b_last = np.zeros((128, 3, 128), np.float32)
    w2b_last[0:64] = w2t[2, :, 2]      # kh=2
    shared["w2bl"] = np.ascontiguousarray(w2b_last, dtype=NPBF16)
    # generic layers
    for (name, Cin, Cout, *_rest) in GEN_LAYERS:
        wd = circ_expand_np(g[f"c{name}"])
        shared[f"w{name}"] = pack_w_generic(wd, Cin // 128, Cout // 128)
    # bn params
    def pk(v, parts):
        v = np.asarray(v, np.float32)
        mch = v.size // parts
        return np.ascontiguousarray(v.reshape(mch, parts).T)
    shared["gn1"] = pk(g["g1"], 64)
    shared["bn1"] = pk(g["b1"], 64)
    for name, c in [("2", 128), ("3a", 256), ("3b", 256), ("4a", 512),
                    ("4b", 512), ("5a", 512), ("5b", 512)]:
        shared[f"gn{name}"] = pk(g[f"g{name}"], 128)
        shared[f"bn{name}"] = pk(g[f"b{name}"], 128)
        assert np.all(g[f"g{name}"] >= 0), "pool/BN commute needs g >= 0"
    assert np.all(g["g1"] >= 0)
    # fc (fold /16 global-mean into weights)
    fcw = (g["fc_w"].T / 16.0)  # (512, 101)
    shared["fcw"] = np.ascontiguousarray(
        fcw.reshape(4, 128, 101).transpose(1, 0, 2), dtype=NPBF16)
    shared["fcb"] = np.ascontiguousarray(g["fc_b"].reshape(101, 1))
    # per-core conv1 im2col
    x = g["x"]  # (8, 3, 16, 112, 112)
    x1_list = []
    for i in range(x.shape[0]):
        xp = np.zeros((3, 18, 114, 114), np.float32)
        xp[:, 1:17, 1:113, 1:113] = x[i]
        sw = np.lib.stride_tricks.sliding_window_view(xp, (3, 3, 3), axis=(1, 2, 3))
        b1 = sw.transpose(0, 4, 5, 6, 1, 2, 3).reshape(81, 16, 12544)
        x1_list.append(np.ascontiguousarray(b1, dtype=NPBF16))
    return shared, x1_list


def build_bass(n_cores, fake_cc=False):
    nc = bacc.Bacc("TRN2", target_bir_lowering=False, debug=False,
                   num_devices=n_cores)
    rg = [list(range(n_cores))]

    din = {}
    din["x1"] = nc.dram_tensor("x1", [81, 16, 12544], BF16, kind="ExternalInput")
    din["w1"] = nc.dram_tensor("w1", [81, 64], BF16, kind="ExternalInput")
    din["w2a"] = nc.dram_tensor("w2a", [128, 9, 128], BF16, kind="ExternalInput")
    din["w2bp"] = nc.dram_tensor("w2bp", [128, 3, 128], BF16, kind="ExternalInput")
    din["w2bl"] = nc.dram_tensor("w2bl", [128, 3, 128], BF16, kind="ExternalInput")
    for (name, Cin, Cout, *_r) in GEN_LAYERS:
        din[f"w{name}"] = nc.dram_tensor(
            f"w{name}", [Cout // 128, 128, Cin // 128, 27, 128], BF16,
            kind="ExternalInput")
    din["gn1"] = nc.dram_tensor("gn1", [64, 1], F32, kind="ExternalInput")
    din["bn1"] = nc.dram_tensor("bn1", [64, 1], F32, kind="ExternalInput")
    for name, c in [("2", 128), ("3a", 256), ("3b", 256), ("4a", 512),
                    ("4b", 512), ("5a", 512), ("5b", 512)]:
        mch = c // 128
        din[f"gn{name}"] = nc.dram_tensor(f"gn{name}", [128, mch], F32,
                                          kind="ExternalInput")
        din[f"bn{name}"] = nc.dram_tensor(f"bn{name}", [128, mch], F32,
                                          kind="ExternalInput")
    din["fcw"] = nc.dram_tensor("fcw", [128, 4, 101], BF16, kind="ExternalInput")
    din["fcb"] = nc.dram_tensor("fcb", [101, 1], F32, kind="ExternalInput")
    logits = nc.dram_tensor("logits", [101, 1], F32, kind="ExternalOutput")

    with tile.TileContext(nc) as tc:
        build_graph(tc, din, logits, rg, fake_cc)
    nc.compile()
    return nc


def build_graph(tc, din, logits, rg, fake_cc=False):
    nc = tc.nc
    import contextlib
    ctx = contextlib.ExitStack()
    inv_n = 1.0 / len(rg[0])
    with ctx:
        singles = ctx.enter_context(tc.tile_pool(name="singles", bufs=1))
        small = ctx.enter_context(tc.tile_pool(name="small", bufs=3))
        statsp = ctx.enter_context(tc.tile_pool(name="statsp", bufs=1))
        psum = ctx.enter_context(tc.tile_pool(name="psum", bufs=3, space="PSUM"))
        psfc = ctx.enter_context(tc.tile_pool(name="psfc", bufs=1, space="PSUM"))
        ybfp = ctx.enter_context(tc.tile_pool(name="ybfp", bufs=3))
        pwp = ctx.enter_context(tc.tile_pool(name="pwp", bufs=3))
        ypoolp = ctx.enter_context(tc.tile_pool(name="ypoolp", bufs=2))
        arena = ctx.enter_context(tc.tile_pool(name="arena", bufs=1))
        dram = ctx.enter_context(tc.tile_pool(name="dram", bufs=1, space="DRAM"))

        # weights first on the DMA queue
        w1_sb = singles.tile([81, 64], BF16, tag="w1")
        nc.sync.dma_start(w1_sb[:], din["w1"][:])
        w2a_sb = singles.tile([128, 9, 128], BF16, tag="w2a")
        nc.sync.dma_start(w2a_sb[:], din["w2a"][:])
        w2bp_sb = singles.tile([128, 3, 128], BF16, tag="w2bp")
        nc.sync.dma_start(w2bp_sb[:], din["w2bp"][:])
        w2bl_sb = singles.tile([128, 3, 128], BF16, tag="w2bl")
        nc.sync.dma_start(w2bl_sb[:], din["w2bl"][:])
        fcw_sb = singles.tile([128, 4, 101], BF16, tag="fcw")
        nc.sync.dma_start(fcw_sb[:], din["fcw"][:])
        fcb_sb = singles.tile([101, 1], F32, tag="fcb")
        nc.sync.dma_start(fcb_sb[:], din["fcb"][:])

        eps_t = singles.tile([128, 1], F32, tag="eps")
        nc.vector.memset(eps_t[:], EPS)

        # persistent small params
        params = {}
        for name, parts in [("1", 64), ("2", 128), ("3a", 128), ("3b", 128),
                            ("4a", 128), ("4b", 128), ("5a", 128), ("5b", 128)]:
            mch = din[f"gn{name}"].shape[1]
            gt = singles.tile([parts, mch], F32, tag=f"g{name}")
            bt = singles.tile([parts, mch], F32, tag=f"b{name}")
            nc.sync.dma_start(gt[:], din[f"gn{name}"][:])
            nc.sync.dma_start(bt[:], din[f"bn{name}"][:])
            params[name] = (gt, bt)

        y1_dram = dram.tile([128, 8, 3136], BF16, tag="y1d")

        def cc_allreduce(tag, cc_sb):
            """AllReduce a small [parts, F] f32 SBUF tile; returns SBUF tile."""
            parts, F = cc_sb.shape
            ccin = dram.tile([parts, F], F32, tag=f"ccin{tag}")
            ccout = dram.tile([parts, F], F32, tag=f"ccout{tag}",
                              addr_space="Shared")
            nc.sync.dma_start(ccin[:], cc_sb[:])
            if fake_cc:
                nc.sync.dma_start(ccout[:], ccin[:])
            else:
                nc.gpsimd.collective_compute(
                    "AllReduce", ADD, replica_groups=rg,
                    ins=[ccin.opt()], outs=[ccout.opt()])
            ar = small.tile([parts, F], F32, tag="ar", name=f"ar{tag}")
            nc.sync.dma_start(ar[:], ccout[:])
            return ar

        def pack_me(stats_view, parts, tag):
            """bn_aggr a stats slice -> [parts, 2] (mean, E2) SBUF tile."""
            mv = small.tile([parts, 2], F32, tag="mv", name=f"mv{tag}")
            nc.vector.bn_aggr(mv[:], stats_view)
            sq = small.tile([parts, 1], F32, tag="sq", name=f"sq{tag}")
            nc.vector.tensor_mul(sq[:], mv[:, 0:1], mv[:, 0:1])
            me = small.tile([parts, 2], F32, tag="me", name=f"me{tag}")
            nc.vector.tensor_copy(me[:, 0:1], mv[:, 0:1])
            nc.vector.tensor_add(me[:, 1:2], mv[:, 1:2], sq[:])
            return me

        def prep_st(mg, e2, gt, bt, parts, tag):
            """From global mean/E2 [parts,1] compute s,t [parts,1]."""
            sq2 = small.tile([parts, 1], F32, tag="sq2", name=f"sq2{tag}")
            nc.vector.tensor_mul(sq2[:], mg[:], mg[:])
            varg = small.tile([parts, 1], F32, tag="varg", name=f"varg{tag}")
            nc.vector.tensor_sub(varg[:], e2[:], sq2[:])
            sd = small.tile([parts, 1], F32, tag="sd", name=f"sd{tag}")
            nc.scalar.activation(sd[:], varg[:], SQRT, bias=eps_t[:parts])
            inv = small.tile([parts, 1], F32, tag="inv", name=f"inv{tag}")
            nc.vector.reciprocal(inv[:], sd[:])
            s_t = small.tile([parts, 1], F32, tag="s_t", name=f"s{tag}")
            nc.vector.tensor_mul(s_t[:], inv[:], gt)
            tmn = small.tile([parts, 1], F32, tag="tmn", name=f"tmn{tag}")
            nc.vector.tensor_mul(tmn[:], mg[:], s_t[:])
            t_t = small.tile([parts, 1], F32, tag="t_t", name=f"t{tag}")
            nc.vector.tensor_sub(t_t[:], bt, tmn[:])
            return s_t, t_t

        # ---------------- conv1 ----------------
        # stats windows: 8 zp x 2 half x 7 t2 = 112 entries of [128, 2, 448]
        stats1 = statsp.tile([128, 224, 6], F32, tag="stats1")
        with tc.tile_pool(name="x1p", bufs=2) as x1p, \
             tc.tile_pool(name="zplp", bufs=2) as zplp, \
             nc.named_scope("conv1"):
            for zp in range(8):
                zplane = zplp.tile([128, 56, 56], BF16, tag="zpl")
                for half in range(2):
                    slab = x1p.tile([81, 2, 6272], BF16, tag="slab")
                    for zg in range(2):
                        src = din["x1"][:, 2 * zp + zg,
                                        half * 6272:(half + 1) * 6272]
                        if zp == 0 and half == 0:
                            # finer first chunks so matmuls start earlier
                            for q in range(4):
                                nc.sync.dma_start(
                                    slab[:, zg, q * 1568:(q + 1) * 1568],
                                    src[:, q * 1568:(q + 1) * 1568])
                        else:
                            nc.sync.dma_start(slab[:, zg], src)
                    for t2 in range(7):
                        pst = psum.tile([128, 2, 512], F32, tag="ps", name="ps")
                        for j in range(2):
                            t = 2 * t2 + j
                            for zg in range(2):
                                nc.tensor.matmul(
                                    pst[zg * 64:(zg + 1) * 64, j, :448],
                                    w1_sb[:],
                                    slab[:, zg, t * 448:(t + 1) * 448],
                                    start=True, stop=True,
                                    tile_position=(0, 64 * zg))
                        ybft = ybfp.tile([128, 896], BF16, tag="ybf",
                                         name="ybf")
                        yb2 = ybft[:].rearrange("p (j n) -> p j n", j=2)
                        nc.scalar.activation(yb2, pst[:, :, :448], COPY)
                        ti = (zp * 2 + half) * 14 + 2 * t2
                        nc.vector.bn_stats(stats1[:, ti], ybft[:, 0:448])
                        nc.vector.bn_stats(stats1[:, ti + 1], ybft[:, 448:896])
                        v = ybft[:].rearrange("p (a b) -> p a b", a=8)
                        pw = pwp.tile([128, 8, 56], BF16, tag="pw")
                        nc.vector.tensor_max(pw[:], v[:, :, 0::2], v[:, :, 1::2])
                        ro = half * 28 + 4 * t2
                        nc.vector.tensor_max(zplane[:, ro:ro + 4, :],
                                             pw[:, 0::2, :], pw[:, 1::2, :])
                nc.sync.dma_start(y1_dram[:, zp, :],
                                  zplane[:].rearrange("p a b -> p (a b)"))
                if zp == 6:
                    with nc.named_scope("ar1A"):
                        meA = pack_me(stats1[:, 0:196], 128, "1A")
                        ar1A = cc_allreduce("1A", meA)
        with nc.named_scope("ar1B"):
            meB = pack_me(stats1[:, 196:224], 128, "1B")
            ar1B = cc_allreduce("1B", meB)
            # combine pieces (weighted by window counts), then partition halves
            wA, wB = 196.0 / 224.0, 28.0 / 224.0
            mh = small.tile([128, 2], F32, tag="mh")  # (mean, E2) per half
            tmpA = small.tile([128, 2], F32, tag="tmpA")
            nc.vector.tensor_scalar_mul(tmpA[:], ar1A[:], wA * inv_n)
            nc.vector.tensor_scalar_mul(mh[:], ar1B[:], wB * inv_n)
            nc.vector.tensor_add(mh[:], mh[:], tmpA[:])
            # cross-half combine: parts 64-127 hold odd-z stats
            mh_hi = small.tile([64, 2], F32, tag="mh_hi")
            nc.sync.dma_start(mh_hi[:], mh[64:128, :])
            mg1 = small.tile([64, 1], F32, tag="mg1")
            e21 = small.tile([64, 1], F32, tag="e21")
            nc.vector.tensor_add(mg1[:], mh[0:64, 0:1], mh_hi[:, 0:1])
            nc.vector.tensor_add(e21[:], mh[0:64, 1:2], mh_hi[:, 1:2])
            nc.vector.tensor_scalar_mul(mg1[:], mg1[:], 0.5)
            nc.vector.tensor_scalar_mul(e21[:], e21[:], 0.5)
            gt1, bt1 = params["1"]
            s1l, t1l = prep_st(mg1, e21, gt1[:, 0:1], bt1[:, 0:1], 64, "1")
            # duplicate to upper partitions for 128-part plane builds
            s1 = small.tile([128, 1], F32, tag="s1full")
            t1 = small.tile([128, 1], F32, tag="t1full")
            nc.sync.dma_start(s1[0:64, :], s1l[:])
            nc.sync.dma_start(s1[64:128, :], s1l[:])
            nc.sync.dma_start(t1[0:64, :], t1l[:])
            nc.sync.dma_start(t1[64:128, :], t1l[:])

        # ---------------- conv2 ----------------
        # stats: 16 z x 4 p2 x 2 j = 128 entries of [128, 392]
        stats2 = statsp.tile([128, 128, 6], F32, tag="stats2")
        conv2_pool = tc.tile_pool(name="c2pool", bufs=1)
        c2pool = conv2_pool.__enter__()
        S2 = c2pool.tile([128, 16, 28, 28], BF16, tag="s2stage")
        with tc.tile_pool(name="plp", bufs=3) as plp, \
             tc.tile_pool(name="b2p", bufs=2) as b2p, \
             tc.tile_pool(name="c2p", bufs=2) as c2p, \
             nc.named_scope("conv2"):

            def load_plane(dst, pidx):
                """DMA y1 plane (padded idx) into a [64,...] partition range."""
                z = pidx - 1
                nc.sync.dma_start(dst, y1_dram[64 * (z % 2):64 * (z % 2) + 64,
                                               z // 2, :])

            def build_b2(B2, z):
                """B2: parts 0-63 = padded plane z, 64-127 = padded plane z+1."""
                p0, p1 = z, z + 1  # padded plane indices
                pl = plp.tile([128, 3136], BF16, tag="pl")
                halves = []
                for hi, pidx in enumerate((p0, p1)):
                    sl = slice(64 * hi, 64 * hi + 64)
                    if pidx == 0 or pidx == 17:
                        nc.vector.memset(B2[sl], 0.0)
                    else:
                        load_plane(pl[sl], pidx)
                        halves.append(sl)
                        nc.vector.memset(B2[sl, 0, :], 0.0)
                        nc.vector.memset(B2[sl, 57, :], 0.0)
                        nc.vector.memset(B2[sl, 1:57, 0:1], 0.0)
                        nc.vector.memset(B2[sl, 1:57, 57:58], 0.0)
                if len(halves) == 2:
                    nc.scalar.activation(
                        B2[:, 1:57, 1:57],
                        pl[:].rearrange("p (a b) -> p a b", a=56),
                        RELU, bias=t1[:, 0:1], scale=s1[:, 0:1])
                elif len(halves) == 1:
                    sl = halves[0]
                    nc.scalar.activation(
                        B2[sl, 1:57, 1:57],
                        pl[sl].rearrange("p (a b) -> p a b", a=56),
                        RELU, bias=t1[sl, 0:1], scale=s1[sl, 0:1])

            def build_c2(C2, z):
                """C2: parts 0-63 = padded plane z+2; parts 64-127 = same
                plane shifted up one row (row y = padded row y+1)."""
                pidx = z + 2
                if pidx == 17:
                    nc.vector.memset(C2[:], 0.0)
                    return
                pl = plp.tile([128, 3136], BF16, tag="pl")
                load_plane(pl[0:64], pidx)
                load_plane(pl[64:128], pidx)
                # lower: normal padded plane
                nc.vector.memset(C2[0:64, 0, :], 0.0)
                nc.vector.memset(C2[0:64, 57, :], 0.0)
                nc.vector.memset(C2[0:64, 1:57, 0:1], 0.0)
                nc.vector.memset(C2[0:64, 1:57, 57:58], 0.0)
                nc.scalar.activation(
                    C2[0:64, 1:57, 1:57],
                    pl[0:64].rearrange("p (a b) -> p a b", a=56),
                    RELU, bias=t1[0:64, 0:1], scale=s1[0:64, 0:1])
                # upper: rows 0..55 = act(plane rows 0..55), rows 56,57 zero,
                # cols 0,57 zero
                nc.vector.memset(C2[64:128, 56:58, :], 0.0)
                nc.vector.memset(C2[64:128, 0:56, 0:1], 0.0)
                nc.vector.memset(C2[64:128, 0:56, 57:58], 0.0)
                nc.scalar.activation(
                    C2[64:128, 0:56, 1:57],
                    pl[64:128].rearrange("p (a b) -> p a b", a=56),
                    RELU, bias=t1[64:128, 0:1], scale=s1[64:128, 0:1])

            for z in range(16):
                B2 = b2p.tile([128, 58, 58], BF16, tag="b2")
                build_b2(B2, z)
                C2 = c2p.tile([128, 58, 58], BF16, tag="c2")
                build_c2(C2, z)
                for p2 in range(4):
                    pst = psum.tile([128, 2, 512], F32, tag="ps", name="ps")
                    for k9 in range(9):
                        kh, kw = k9 // 3, k9 % 3
                        for j in range(2):
                            y0 = 14 * p2 + 7 * j + kh
                            nc.tensor.matmul(pst[:, j, :392], w2a_sb[:, k9, :],
                                             B2[:, y0:y0 + 7, kw:kw + 56],
                                             start=(k9 == 0), stop=False)
                    for kw in range(3):
                        for j in range(2):
                            y0 = 14 * p2 + 7 * j
                            nc.tensor.matmul(pst[:, j, :392], w2bp_sb[:, kw, :],
                                             C2[:, y0:y0 + 7, kw:kw + 56],
                                             start=False, stop=False)
                    for kw in range(3):
                        for j in range(2):
                            y0 = 14 * p2 + 7 * j + 2
                            nc.tensor.matmul(pst[:, j, :392], w2bl_sb[:, kw, :],
                                             C2[:, y0:y0 + 7, kw:kw + 56],
                                             start=False,
                                             stop=(kw == 2 and j == 1))
                    ybft = ybfp.tile([128, 784], BF16, tag="ybf2",
                                     name="ybf")
                    yb2 = ybft[:].rearrange("p (j n) -> p j n", j=2)
                    nc.vector.tensor_copy(yb2, pst[:, :, :392])
                    ti = z * 8 + 2 * p2
                    nc.vector.bn_stats(stats2[:, ti], ybft[:, 0:392])
                    nc.vector.bn_stats(stats2[:, ti + 1], ybft[:, 392:784])
                    v = ybft[:].rearrange("p (a b) -> p a b", a=14)
                    pw = pwp.tile([128, 14, 28], BF16, tag="pw2")
                    nc.vector.tensor_max(pw[:], v[:, :, 0::2], v[:, :, 1::2])
                    nc.vector.tensor_max(S2[:, z, 7 * p2:7 * p2 + 7, :],
                                         pw[:, 0::2, :], pw[:, 1::2, :])
                if z == 13:
                    with nc.named_scope("ar2A"):
                        meA = pack_me(stats2[:, 0:112], 128, "2A")
                        ar2A = cc_allreduce("2A", meA)
        with nc.named_scope("ar2B"):
            meB = pack_me(stats2[:, 112:128], 128, "2B")
            ar2B = cc_allreduce("2B", meB)
            wA, wB = 112.0 / 128.0, 16.0 / 128.0
            me2 = small.tile([128, 2], F32, tag="me2g")
            tmp2 = small.tile([128, 2], F32, tag="tmp2g")
            nc.vector.tensor_scalar_mul(tmp2[:], ar2A[:], wA * inv_n)
            nc.vector.tensor_scalar_mul(me2[:], ar2B[:], wB * inv_n)
            nc.vector.tensor_add(me2[:], me2[:], tmp2[:])
            gt2, bt2 = params["2"]
            s2, t2 = prep_st(me2[:, 0:1], me2[:, 1:2], gt2[:, 0:1], bt2[:, 0:1],
                             128, "2")
        Y2p = c2pool.tile([128, 8, 28, 28], BF16, tag="y2pool")
        nc.vector.tensor_max(Y2p[:], S2[:, 0::2], S2[:, 1::2])
        P3in = arena.tile([128, 10, 30, 30], BF16, tag="pin3a")

        def zero_borders1(P, Dp, Hp, Wp):
            nc.vector.memset(P[:, 0], 0.0)
            nc.vector.memset(P[:, Dp - 1], 0.0)
            nc.vector.memset(P[:, 1:Dp - 1, 0, :], 0.0)
            nc.vector.memset(P[:, 1:Dp - 1, Hp - 1, :], 0.0)
            nc.vector.memset(P[:, 1:Dp - 1, 1:Hp - 1, 0:1], 0.0)
            nc.vector.memset(P[:, 1:Dp - 1, 1:Hp - 1, Wp - 1:Wp], 0.0)

        zero_borders1(P3in, 10, 30, 30)
        nc.scalar.activation(P3in[:, 1:9, 1:29, 1:29], Y2p[:], RELU,
                             bias=t2[:, 0:1], scale=s2[:, 0:1])
        conv2_pool.__exit__(None, None, None)

        # ---------------- generic conv layers ----------------
        with tc.tile_pool(name="wp", bufs=2) as wp, \
             tc.tile_pool(name="stagep", bufs=2) as stagep:
            pins = [P3in]
            for (name, Cin, Cout, D, H, W, R, zpair, pooled) in GEN_LAYERS:
                Kch, Mch = Cin // 128, Cout // 128
                ntz = D // 2 if zpair else D
                zcnt = 2 if zpair else 1
                ytiles = H // R
                N = zcnt * R * W
                T_chunk = ntz * ytiles
                H2, W2, D2 = H // 2, W // 2, D // 2
                gt, bt = params[name]
                tiles = [(2 * tz if zpair else tz, ty * R)
                         for tz in range(ntz) for ty in range(ytiles)]
                groups = [tiles[i:i + 2] for i in range(0, len(tiles), 2)]
                next_pins = []
                feats = None
                if name == "5b":
                    feats = small.tile([128, 4], F32, tag="feats")
                stage_tag = "stage3a" if name == "3a" else "stageS"
                for m in range(Mch):
                    scope = nc.named_scope(f"conv{name}_{m}")
                    scope.__enter__()
                    stats_m = statsp.tile([128, T_chunk, 6], F32, tag="stats_g",
                                          name=f"st{name}{m}", bufs=3)
                    if pooled is False or pooled is None:
                        stage = stagep.tile([128, D, H, W], BF16, tag=stage_tag,
                                            name=f"stage{name}{m}")
                    else:
                        stage = stagep.tile([128, D, H2, W2], BF16,
                                            tag=stage_tag,
                                            name=f"stage{name}{m}")
                    wm = wp.tile([128, Kch, 27, 128], BF16, tag="w",
                                 name=f"w{name}{m}")
                    nc.sync.dma_start(wm[:, :Kch], din[f"w{name}"][m])
                    ti = 0
                    for grp in groups:
                        G = len(grp)
                        pst = psum.tile([128, 2, 512], F32, tag="ps",
                                        name="ps")
                        nmm = Kch * 27
                        i = 0
                        for c in range(Kch):
                            for (kd, kh, kw) in TAPS:
                                for j, (z0, y0) in enumerate(grp):
                                    rhs = pins[c][:, z0 + kd:z0 + kd + zcnt,
                                                  y0 + kh:y0 + kh + R,
                                                  kw:kw + W]
                                    nc.tensor.matmul(
                                        pst[:, j, :N],
                                        wm[:, c, kd * 9 + kh * 3 + kw, :],
                                        rhs, start=(i == 0), stop=(i == nmm - 1))
                                i += 1
                        z0, y0 = grp[0]
                        if pooled is False or pooled is None:
                            # write raw conv outputs into stage
                            dst = stage[:].rearrange("p d h w -> p (d h w)")
                            off = z0 * H * W + y0 * W
                            nc.vector.tensor_copy(
                                dst[:, off:off + G * N].rearrange(
                                    "p (g n) -> p g n", g=G),
                                pst[:, :G, :N])
                            for j in range(G):
                                nc.vector.bn_stats(
                                    stats_m[:, ti + j],
                                    dst[:, off + j * N:off + (j + 1) * N])
                        else:
                            ybft = ybfp.tile([128, 784], BF16,
                                             tag="ybf2", name="ybf")
                            ybf = ybft[:, :G * N]
                            nc.vector.tensor_copy(
                                ybf.rearrange("p (g n) -> p g n", g=G),
                                pst[:, :G, :N])
                            for j in range(G):
                                nc.vector.bn_stats(
                                    stats_m[:, ti + j],
                                    ybft[:, j * N:(j + 1) * N])
                            nz = G * zcnt if zpair else 1
                            nr = R if zpair else G * R
                            v = ybf.rearrange("p (z y x) -> p z y x",
                                              z=nz, y=nr)
                            pw = pwp.tile([128, nz, nr, W2], BF16,
                                          tag="pw3", name="pw")
                            nc.vector.tensor_max(pw[:], v[:, :, :, 0::2],
                                                 v[:, :, :, 1::2])
                            nc.vector.tensor_max(
                                stage[:, z0:z0 + nz,
                                      y0 // 2:y0 // 2 + nr // 2, :],
                                pw[:, :, 0::2, :], pw[:, :, 1::2, :])
                        ti += G
                    scope.__exit__(None, None, None)
                    # ---- per-chunk stats reduce + BN prep + apply ----
                    with nc.named_scope(f"ar{name}_{m}"):
                        me = pack_me(stats_m[:], 128, f"{name}{m}")
                        ar = cc_allreduce(f"{name}{m}", me)
                        mg = small.tile([128, 1], F32, tag="mgc",
                                        name=f"mg{name}{m}")
                        e2 = small.tile([128, 1], F32, tag="e2c",
                                        name=f"e2{name}{m}")
                        nc.vector.tensor_scalar_mul(mg[:], ar[:, 0:1], inv_n)
                        nc.vector.tensor_scalar_mul(e2[:], ar[:, 1:2], inv_n)
                        s_m, t_m = prep_st(mg, e2, gt[:, m:m + 1], bt[:, m:m + 1],
                                           128, f"{name}{m}")

                    if name == "5b":
                        # pool5: window (2,2,2) stride 2, pad (0,1,1), then
                        # BN+ReLU and global mean (1/16 folded into fcw)
                        pd = small.tile([128, 7, 7], BF16, tag="pd5",
                                        name=f"pd5{m}")
                        nc.vector.tensor_max(pd[:], stage[:, 0], stage[:, 1])
                        pw5 = small.tile([128, 7, 4], BF16, tag="pw5",
                                         name=f"pw5{m}")
                        nc.vector.tensor_copy(pw5[:, :, 0:1], pd[:, :, 0:1])
                        nc.vector.tensor_max(pw5[:, :, 1:4],
                                             pd[:, :, 1::2], pd[:, :, 2::2])
                        ph5 = small.tile([128, 4, 4], BF16, tag="ph5",
                                         name=f"ph5{m}")
                        nc.vector.tensor_copy(ph5[:, 0:1, :], pw5[:, 0:1, :])
                        nc.vector.tensor_max(ph5[:, 1:4, :],
                                             pw5[:, 1::2, :], pw5[:, 2::2, :])
                        Z = small.tile([128, 16], BF16, tag="z5",
                                       name=f"z5{m}")
                        nc.scalar.activation(
                            Z[:], ph5[:].rearrange("p a b -> p (a b)"),
                            RELU, bias=t_m[:], scale=s_m[:])
                        nc.vector.tensor_reduce(feats[:, m:m + 1], Z[:],
                                                axis=mybir.AxisListType.X,
                                                op=ADD)
                        continue

                    if pooled:
                        src = ypoolp.tile([128, D2, H2, W2], BF16, tag="ypool",
                                          name=f"yp{name}{m}")
                        nc.vector.tensor_max(src[:], stage[:, 0::2],
                                             stage[:, 1::2])
                        nD, nH, nW = D2, H2, W2
                    else:
                        src = stage
                        nD, nH, nW = D, H, W
                    Pn = arena.tile([128, nD + 2, nH + 2, nW + 2], BF16,
                                    tag=f"pin_{name}", name=f"pin{name}{m}",
                                    bufs=Mch)
                    zero_borders1(Pn, nD + 2, nH + 2, nW + 2)
                    nc.scalar.activation(
                        Pn[:, 1:1 + nD, 1:1 + nH, 1:1 + nW],
                        src[:], RELU, bias=t_m[:], scale=s_m[:])
                    next_pins.append(Pn)
                pins = next_pins

            # FC tail
            fcin = small.tile([128, 4], BF16, tag="fcin")
            nc.vector.tensor_copy(fcin[:], feats[:])
            psf = psfc.tile([101, 1], F32, tag="psfc")
            for c in range(4):
                nc.tensor.matmul(psf[:], fcw_sb[:, c, :],
                                 fcin[:, c:c + 1],
                                 start=(c == 0), stop=(c == 3))
            out_sb = small.tile([101, 1], F32, tag="outsb")
            nc.scalar.activation(out_sb[:], psf[:], IDENT,
                                 bias=fcb_sb[:])
            nc.sync.dma_start(logits[:], out_sb[:])


_STATE = {}


def _get_nc(n_cores=N_CORES):
    key = f"nc{n_cores}"
    if key not in _STATE:
        _STATE[key] = build_bass(n_cores)
    return _STATE[key]


def kernel(**inputs):
    nc = _get_nc()
    shared, x1_list = host_prep(inputs)
    in_maps = []
    for i in range(N_CORES):
        m = dict(shared)
        m["x1"] = x1_list[i]
        in_maps.append(m)
    res = run_bass_kernel_spmd(nc, in_maps, core_ids=list(range(N_CORES)))
    out = np.stack([res.results[i]["logits"].reshape(101)
                    for i in range(N_CORES)]).astype(np.float32)
    return out


# revision 44
# speedup vs baseline: 1.0321x; 1.0321x over previous
"""C3D-style circulant-block 3D CNN forward pass on 8 Trainium2 NeuronCores.

Sharding: data-parallel over batch (8 samples -> 8 cores). Training-mode
BatchNorm batch statistics are combined across cores with small per-chunk
f32 AllReduces of (mean, E[x^2]) per channel, issued as soon as each
output-channel chunk (or spatial piece, for single-chunk layers) finishes
so the collective overlaps the remaining matmuls.

Device kernel per core (per sample):
  conv1 via host-side im2col (K=81) -> col-tiled matmuls that compute two
  z-planes concurrently (output channels 0-63 on partitions 0-63 for even
  z, 64-127 for odd z). PSUM->bf16 on ACT, batched bn_stats on DVE, both
  maxpool stages on GpSimd.
  conv2: input channels x kd{0,1} packed into K=128 (B2) + kd=2 handled by
  a row-shift-duplicated plane (C2: parts 64-127 hold the plane shifted by
  one row) so kh taps pair up -> 15 matmuls per tile instead of 18.
  conv3a..5b as shift-and-accumulate implicit GEMM over per-chunk padded
  SBUF inputs; per chunk: stats -> AllReduce -> BN prep -> pool -> fused
  BN+ReLU apply into the next layer's padded chunk, all overlapped with
  the next chunk's matmuls.
  Tail: per-chunk special pool5 -> global mean (folded /16 into FC
  weights) -> FC matmul -> logits.
"""

import numpy as np
import ml_dtypes

import concourse.bass as bass
import concourse.mybir as mybir
import concourse.tile as tile
from concourse import bacc
from concourse.bass_utils import run_bass_kernel_spmd

F32 = mybir.dt.float32
BF16 = mybir.dt.bfloat16
NPBF16 = ml_dtypes.bfloat16
RELU = mybir.ActivationFunctionType.Relu
COPY = mybir.ActivationFunctionType.Copy
SQRT = mybir.ActivationFunctionType.Sqrt
IDENT = mybir.ActivationFunctionType.Identity
ADD = mybir.AluOpType.add
EPS = 1e-5
N_CORES = 8

# name, Cin, Cout, D, H, W, R(rows/tile), zpair, pooled
GEN_LAYERS = [
    ("3a", 128, 256, 8, 28, 28, 14, False, False),
    ("3b", 256, 256, 8, 28, 28, 14, False, True),
    ("4a", 256, 512, 4, 14, 14, 14, True, False),
    ("4b", 512, 512, 4, 14, 14, 14, True, True),
    ("5a", 512, 512, 2, 7, 7, 7, True, False),
    ("5b", 512, 512, 2, 7, 7, 7, True, None),  # None -> raw stage (special pool)
]

TAPS = [(kd, kh, kw) for kd in range(3) for kh in range(3) for kw in range(3)]


def circ_expand_np(c):
    c = np.asarray(c, np.float32)
    P, Q, b = c.shape[0], c.shape[1], c.shape[2]
    r = np.arange(b)
    idx = (r[:, None] - r[None, :]) % b
    w = c[:, :, idx]  # (P, Q, b, b, k, k, k)
    w = np.transpose(w, (0, 2, 1, 3, 4, 5, 6))
    return w.reshape(P * b, Q * b, *c.shape[3:])


def pack_w_generic(wd, Kch, Mch):
    # wd (Co, Ci, 3,3,3) -> [Mch, 128ci, Kch, 27, 128co] bf16
    Co, Ci = wd.shape[0], wd.shape[1]
    wt = wd.transpose(1, 2, 3, 4, 0)  # (Ci, kd,kh,kw, Co)
    wt = wt.reshape(Kch, 128, 27, Mch, 128)
    wt = wt.transpose(3, 1, 0, 2, 4)  # (m, ci, c, t, co)
    return np.ascontiguousarray(wt, dtype=NPBF16)


def host_prep(inputs):
    g = {k: np.asarray(v, np.float32) for k, v in inputs.items()}
    shared = {}
    # conv1
    w1 = g["conv1_w"]  # (64, 3, 3,3,3)
    shared["w1"] = np.ascontiguousarray(
        w1.transpose(1, 2, 3, 4, 0).reshape(81, 64), dtype=NPBF16)
    # conv2 (kd-packed + kd2 row-shift-paired)
    w2 = circ_expand_np(g["c2"])  # (128, 64, 3,3,3)
    w2t = w2.transpose(2, 1, 3, 4, 0)  # (kd, ci, kh, kw, co)
    shared["w2a"] = np.ascontiguousarray(
        w2t[0:2].reshape(128, 9, 128), dtype=NPBF16)
    # kd=2: pair (kh0,kh1) via row-shifted upper partitions; kh2 alone with
    # zeroed upper-partition weights.
    w2b_pair = np.zeros((128, 3, 128), np.float32)
    w2b_pair[0:64] = w2t[2, :, 0]      # (ci, kw, co) for kh=0
    w2b_pair[64:128] = w2t[2, :, 1]    # kh=1 (upper = plane shifted +1 row)
    shared["w2bp"] = np.ascontiguousarray(w2b_pair, dtype=NPBF16)
    w2b_last = np.zeros((128, 3, 128), np.float32)
    w2b_last[0:64] = w2t[2, :, 2]      # kh=2
    shared["w2bl"] = np.ascontiguousarray(w2b_last, dtype=NPBF16)
    # generic layers
    for (name, Cin, Cout, *_rest) in GEN_LAYERS:
        wd = circ_expand_np(g[f"c{name}"])
        shared[f"w{name}"] = pack_w_generic(wd, Cin // 128, Cout // 128)
    # bn params
    def pk(v, parts):
        v = np.asarray(v, np.float32)
        mch = v.size // parts
        return np.ascontiguousarray(v.reshape(mch, parts).T)
    shared["gn1"] = pk(g["g1"], 64)
    shared["bn1"] = pk(g["b1"], 64)
    for name, c in [("2", 128), ("3a", 256), ("3b", 256), ("4a", 512),
                    ("4b", 512), ("5a", 512), ("5b", 512)]:
        shared[f"gn{name}"] = pk(g[f"g{name}"], 128)
        shared[f"bn{name}"] = pk(g[f"b{name}"], 128)
        assert np.all(g[f"g{name}"] >= 0), "pool/BN commute needs g >= 0"
    assert np.all(g["g1"] >= 0)
    # fc (fold /16 global-mean into weights)
    fcw = (g["fc_w"].T / 16.0)  # (512, 101)
    shared["fcw"] = np.ascontiguousarray(
        fcw.reshape(4, 128, 101).transpose(1, 0, 2), dtype=NPBF16)
    shared["fcb"] = np.ascontiguousarray(g["fc_b"].reshape(101, 1))
    # per-core conv1 im2col
    x = g["x"]  # (8, 3, 16, 112, 112)
    x1_list = []
    for i in range(x.shape[0]):
        xp = np.zeros((3, 18, 114, 114), np.float32)
        xp[:, 1:17, 1:113, 1:113] = x[i]
        sw = np.lib.stride_tricks.sliding_window_view(xp, (3, 3, 3), axis=(1, 2, 3))
        b1 = sw.transpose(0, 4, 5, 6, 1, 2, 3).reshape(81, 16, 12544)
        x1_list.append(np.ascontiguousarray(b1, dtype=NPBF16))
    return shared, x1_list


def build_bass(n_cores, fake_cc=False):
    nc = bacc.Bacc("TRN2", target_bir_lowering=False, debug=False,
                   num_devices=n_cores)
    rg = [list(range(n_cores))]

    din = {}
    din["x1"] = nc.dram_tensor("x1", [81, 16, 12544], BF16, kind="ExternalInput")
    din["w1"] = nc.dram_tensor("w1", [81, 64], BF16, kind="ExternalInput")
    din["w2a"] = nc.dram_tensor("w2a", [128, 9, 128], BF16, kind="ExternalInput")
    din["w2bp"] = nc.dram_tensor("w2bp", [128, 3, 128], BF16, kind="ExternalInput")
    din["w2bl"] = nc.dram_tensor("w2bl", [128, 3, 128], BF16, kind="ExternalInput")
    for (name, Cin, Cout, *_r) in GEN_LAYERS:
        din[f"w{name}"] = nc.dram_tensor(
            f"w{name}", [Cout // 128, 128, Cin // 128, 27, 128], BF16,
            kind="ExternalInput")
    din["gn1"] = nc.dram_tensor("gn1", [64, 1], F32, kind="ExternalInput")
    din["bn1"] = nc.dram_tensor("bn1", [64, 1], F32, kind="ExternalInput")
    for name, c in [("2", 128), ("3a", 256), ("3b", 256), ("4a", 512),
                    ("4b", 512), ("5a", 512), ("5b", 512)]:
        mch = c // 128
        din[f"gn{name}"] = nc.dram_tensor(f"gn{name}", [128, mch], F32,
                                          kind="ExternalInput")
        din[f"bn{name}"] = nc.dram_tensor(f"bn{name}", [128, mch], F32,
                                          kind="ExternalInput")
    din["fcw"] = nc.dram_tensor("fcw", [128, 4, 101], BF16, kind="ExternalInput")
    din["fcb"] = nc.dram_tensor("fcb", [101, 1], F32, kind="ExternalInput")
    logits = nc.dram_tensor("logits", [101, 1], F32, kind="ExternalOutput")

    with tile.TileContext(nc) as tc:
        build_graph(tc, din, logits, rg, fake_cc)
    nc.compile()
    return nc


def build_graph(tc, din, logits, rg, fake_cc=False):
    nc = tc.nc
    import contextlib
    ctx = contextlib.ExitStack()
    inv_n = 1.0 / len(rg[0])
    with ctx:
        singles = ctx.enter_context(tc.tile_pool(name="singles", bufs=1))
        small = ctx.enter_context(tc.tile_pool(name="small", bufs=3))
        statsp = ctx.enter_context(tc.tile_pool(name="statsp", bufs=1))
        psum = ctx.enter_context(tc.tile_pool(name="psum", bufs=3, space="PSUM"))
        psfc = ctx.enter_context(tc.tile_pool(name="psfc", bufs=1, space="PSUM"))
        ybfp = ctx.enter_context(tc.tile_pool(name="ybfp", bufs=3))
        pwp = ctx.enter_context(tc.tile_pool(name="pwp", bufs=3))
        ypoolp = ctx.enter_context(tc.tile_pool(name="ypoolp", bufs=2))
        dram = ctx.enter_context(tc.tile_pool(name="dram", bufs=1, space="DRAM"))

        # weights first on the DMA queue
        w1_sb = singles.tile([81, 64], BF16, tag="w1")
        nc.sync.dma_start(w1_sb[:], din["w1"][:])
        w2a_sb = singles.tile([128, 9, 128], BF16, tag="w2a")
        nc.sync.dma_start(w2a_sb[:], din["w2a"][:])
        w2bp_sb = singles.tile([128, 3, 128], BF16, tag="w2bp")
        nc.sync.dma_start(w2bp_sb[:], din["w2bp"][:])
        w2bl_sb = singles.tile([128, 3, 128], BF16, tag="w2bl")
        nc.sync.dma_start(w2bl_sb[:], din["w2bl"][:])
        fcw_sb = singles.tile([128, 4, 101], BF16, tag="fcw")
        nc.sync.dma_start(fcw_sb[:], din["fcw"][:])
        fcb_sb = singles.tile([101, 1], F32, tag="fcb")
        nc.sync.dma_start(fcb_sb[:], din["fcb"][:])

        eps_t = singles.tile([128, 1], F32, tag="eps")
        nc.vector.memset(eps_t[:], EPS)

        # conv1 bn params duplicated to both partition halves
        g1f = singles.tile([128, 1], F32, tag="g1f")
        b1f = singles.tile([128, 1], F32, tag="b1f")
        nc.sync.dma_start(g1f[0:64, :], din["gn1"][:])
        nc.sync.dma_start(g1f[64:128, :], din["gn1"][:])
        nc.sync.dma_start(b1f[0:64, :], din["bn1"][:])
        nc.sync.dma_start(b1f[64:128, :], din["bn1"][:])

        # persistent small params
        params = {}
        for name, parts in [("1", 64), ("2", 128), ("3a", 128), ("3b", 128),
                            ("4a", 128), ("4b", 128), ("5a", 128), ("5b", 128)]:
            mch = din[f"gn{name}"].shape[1]
            gt = singles.tile([parts, mch], F32, tag=f"g{name}")
            bt = singles.tile([parts, mch], F32, tag=f"b{name}")
            nc.sync.dma_start(gt[:], din[f"gn{name}"][:])
            nc.sync.dma_start(bt[:], din[f"bn{name}"][:])
            params[name] = (gt, bt)

        y1_dram = dram.tile([128, 8, 3136], BF16, tag="y1d")

        def cc_allreduce(tag, cc_sb):
            """AllReduce a small [parts, F] f32 SBUF tile; returns SBUF tile."""
            parts, F = cc_sb.shape
            ccin = dram.tile([parts, F], F32, tag=f"ccin{tag}")
            ccout = dram.tile([parts, F], F32, tag=f"ccout{tag}",
                              addr_space="Shared")
            nc.sync.dma_start(ccin[:], cc_sb[:])
            if fake_cc:
                nc.sync.dma_start(ccout[:], ccin[:])
            else:
                nc.gpsimd.collective_compute(
                    "AllReduce", ADD, replica_groups=rg,
                    ins=[ccin.opt()], outs=[ccout.opt()])
            ar = small.tile([parts, F], F32, tag="ar", name=f"ar{tag}")
            nc.sync.dma_start(ar[:], ccout[:])
            return ar

        def pack_me(stats_view, parts, tag):
            """bn_aggr a stats slice -> [parts, 2] (mean, E2) SBUF tile."""
            mv = small.tile([parts, 2], F32, tag="mv", name=f"mv{tag}")
            nc.vector.bn_aggr(mv[:], stats_view)
            sq = small.tile([parts, 1], F32, tag="sq", name=f"sq{tag}")
            nc.vector.tensor_mul(sq[:], mv[:, 0:1], mv[:, 0:1])
            me = small.tile([parts, 2], F32, tag="me", name=f"me{tag}")
            nc.vector.tensor_copy(me[:, 0:1], mv[:, 0:1])
            nc.vector.tensor_add(me[:, 1:2], mv[:, 1:2], sq[:])
            return me

        def prep_st(mg, e2, gt, bt, parts, tag, F=1):
            """From global mean/E2 [parts,F] compute s,t [parts,F]."""
            sq2 = small.tile([parts, F], F32, tag=f"sq2w{F}", name=f"sq2{tag}")
            nc.vector.tensor_mul(sq2[:], mg[:], mg[:])
            varg = small.tile([parts, F], F32, tag=f"vargw{F}",
                              name=f"varg{tag}")
            nc.vector.tensor_sub(varg[:], e2[:], sq2[:])
            sd = small.tile([parts, F], F32, tag=f"sdw{F}", name=f"sd{tag}")
            nc.scalar.activation(sd[:], varg[:], SQRT, bias=eps_t[:parts])
            inv = small.tile([parts, F], F32, tag=f"invw{F}", name=f"inv{tag}")
            nc.vector.reciprocal(inv[:], sd[:])
            s_t = small.tile([parts, F], F32, tag=f"s_tw{F}", name=f"s{tag}")
            nc.vector.tensor_mul(s_t[:], inv[:], gt)
            tmn = small.tile([parts, F], F32, tag=f"tmnw{F}", name=f"tmn{tag}")
            nc.vector.tensor_mul(tmn[:], mg[:], s_t[:])
            t_t = small.tile([parts, F], F32, tag=f"t_tw{F}", name=f"t{tag}")
            nc.vector.tensor_sub(t_t[:], bt, tmn[:])
            return s_t, t_t

        # ---------------- conv1 ----------------
        # stats windows: 8 zp x 2 half x 7 t2 = 112 entries of [128, 2, 448]
        stats1 = statsp.tile([128, 224, 6], F32, tag="stats1")
        # 3-way stats split: A (zp0-4, absorbs initial core skew while zp5-7
        # compute), B (zp5-6), C (zp7, exposed ~wire latency only)
        PCS = [(0, 140, "1A", 4), (140, 196, "1B", 6), (196, 224, "1C", None)]
        ars1 = {}
        with tc.tile_pool(name="x1p", bufs=3) as x1p, \
             tc.tile_pool(name="zplp", bufs=2) as zplp, \
             nc.named_scope("conv1"):
            for zp in range(8):
                zplane = zplp.tile([128, 56, 56], BF16, tag="zpl")
                for half in range(2):
                    slab = x1p.tile([81, 2, 6272], BF16, tag="slab")
                    for zg in range(2):
                        src = din["x1"][:, 2 * zp + zg,
                                        half * 6272:(half + 1) * 6272]
                        if zp == 0 and half == 0:
                            # finer first chunks so matmuls start earlier
                            for q in range(4):
                                nc.sync.dma_start(
                                    slab[:, zg, q * 1568:(q + 1) * 1568],
                                    src[:, q * 1568:(q + 1) * 1568])
                        else:
                            nc.sync.dma_start(slab[:, zg], src)
                    ybfh = x1p.tile([128, 6272], BF16, tag="ybf",
                                    name="ybfh", bufs=2)
                    for t2 in range(7):
                        pst = psum.tile([128, 2, 512], F32, tag="ps", name="ps")
                        for j in range(2):
                            t = 2 * t2 + j
                            for zg in range(2):
                                nc.tensor.matmul(
                                    pst[zg * 64:(zg + 1) * 64, j, :448],
                                    w1_sb[:],
                                    slab[:, zg, t * 448:(t + 1) * 448],
                                    start=True, stop=True,
                                    tile_position=(0, 64 * zg))
                        yb2 = ybfh[:, t2 * 896:(t2 + 1) * 896].rearrange(
                            "p (j n) -> p j n", j=2)
                        nc.scalar.activation(yb2, pst[:, :, :448], COPY)
                        ti = (zp * 2 + half) * 14 + 2 * t2
                        nc.vector.bn_stats(stats1[:, ti],
                                           ybfh[:, t2 * 896:t2 * 896 + 448])
                        nc.vector.bn_stats(stats1[:, ti + 1],
                                           ybfh[:, t2 * 896 + 448:(t2 + 1) * 896])
                    # batched pools: one x-pool + one y-pool per half
                    vh = ybfh[:].rearrange("p (a b) -> p a b", a=56)
                    pwh = x1p.tile([128, 56, 56], BF16, tag="pw", bufs=2)
                    nc.vector.tensor_max(pwh[:], vh[:, :, 0::2], vh[:, :, 1::2])
                    nc.vector.tensor_max(zplane[:, half * 28:half * 28 + 28, :],
                                         pwh[:, 0::2, :], pwh[:, 1::2, :])
                nc.sync.dma_start(y1_dram[:, zp, :],
                                  zplane[:].rearrange("p a b -> p (a b)"))
                for (a, b, nm, at_zp) in PCS:
                    if at_zp == zp:
                        with nc.named_scope(f"ar{nm}"):
                            me = pack_me(stats1[:, a:b], 128, nm)
                            ars1[nm] = cc_allreduce(nm, me)
        with nc.named_scope("ar1C"):
            a, b, nm, _ = PCS[-1]
            me = pack_me(stats1[:, a:b], 128, nm)
            ars1[nm] = cc_allreduce(nm, me)
            # combine pieces (weighted by window counts)
            mh = small.tile([128, 2], F32, tag="mh")  # (mean, E2) per half
            tmpA = small.tile([128, 2], F32, tag="tmpA")
            nc.vector.tensor_scalar_mul(mh[:], ars1["1A"][:],
                                        (140.0 / 224.0) * inv_n)
            nc.vector.tensor_scalar_mul(tmpA[:], ars1["1B"][:],
                                        (56.0 / 224.0) * inv_n)
            nc.vector.tensor_add(mh[:], mh[:], tmpA[:])
            nc.vector.tensor_scalar_mul(tmpA[:], ars1["1C"][:],
                                        (28.0 / 224.0) * inv_n)
            nc.vector.tensor_add(mh[:], mh[:], tmpA[:])
            # cross-half combine via partition swap (parts 64-127 = odd z)
            mhs = small.tile([128, 2], F32, tag="mhs")
            nc.sync.dma_start(mhs[0:64, :], mh[64:128, :])
            nc.sync.dma_start(mhs[64:128, :], mh[0:64, :])
            me1 = small.tile([128, 2], F32, tag="me1g")
            nc.vector.tensor_add(me1[:], mh[:], mhs[:])
            nc.vector.tensor_scalar_mul(me1[:], me1[:], 0.5)
            s1, t1 = prep_st(me1[:, 0:1], me1[:, 1:2], g1f[:, 0:1], b1f[:, 0:1],
                             128, "1")

        # ---------------- conv2 ----------------
        # stats: 16 z x 4 p2 x 2 j = 128 entries of [128, 392]
        stats2 = statsp.tile([128, 128, 6], F32, tag="stats2")
        arena = ctx.enter_context(tc.tile_pool(name="arena", bufs=1))
        conv2_pool = tc.tile_pool(name="c2pool", bufs=1)
        c2pool = conv2_pool.__enter__()
        Y2p = c2pool.tile([128, 8, 28, 28], BF16, tag="y2pool")
        with tc.tile_pool(name="plp", bufs=2) as plp, \
             tc.tile_pool(name="b2p", bufs=2) as b2p, \
             tc.tile_pool(name="c2p", bufs=2) as c2p, \
             nc.named_scope("conv2"):

            def load_plane(dst, pidx):
                """DMA y1 plane (padded idx) into a [64,...] partition range."""
                z = pidx - 1
                nc.sync.dma_start(dst, y1_dram[64 * (z % 2):64 * (z % 2) + 64,
                                               z // 2, :])

            def build_b2(B2, z):
                """B2: parts 0-63 = padded plane z, 64-127 = padded plane z+1."""
                p0, p1 = z, z + 1  # padded plane indices
                pl = plp.tile([128, 3136], BF16, tag="pl")
                halves = []
                for hi, pidx in enumerate((p0, p1)):
                    sl = slice(64 * hi, 64 * hi + 64)
                    if pidx == 0 or pidx == 17:
                        nc.vector.memset(B2[sl], 0.0)
                    else:
                        load_plane(pl[sl], pidx)
                        halves.append(sl)
                        nc.vector.memset(B2[sl, 0, :], 0.0)
                        nc.vector.memset(B2[sl, 57, :], 0.0)
                        nc.vector.memset(B2[sl, 1:57, 0:1], 0.0)
                        nc.vector.memset(B2[sl, 1:57, 57:58], 0.0)
                if len(halves) == 2:
                    nc.scalar.activation(
                        B2[:, 1:57, 1:57],
                        pl[:].rearrange("p (a b) -> p a b", a=56),
                        RELU, bias=t1[:, 0:1], scale=s1[:, 0:1])
                elif len(halves) == 1:
                    sl = halves[0]
                    nc.scalar.activation(
                        B2[sl, 1:57, 1:57],
                        pl[sl].rearrange("p (a b) -> p a b", a=56),
                        RELU, bias=t1[sl, 0:1], scale=s1[sl, 0:1])

            def build_c2(C2, z):
                """C2: parts 0-63 = padded plane z+2; parts 64-127 = same
                plane shifted up one row (row y = padded row y+1)."""
                pidx = z + 2
                if pidx == 17:
                    nc.vector.memset(C2[:], 0.0)
                    return
                pl = plp.tile([128, 3136], BF16, tag="pl")
                load_plane(pl[0:64], pidx)
                load_plane(pl[64:128], pidx)
                # lower: normal padded plane
                nc.vector.memset(C2[0:64, 0, :], 0.0)
                nc.vector.memset(C2[0:64, 57, :], 0.0)
                nc.vector.memset(C2[0:64, 1:57, 0:1], 0.0)
                nc.vector.memset(C2[0:64, 1:57, 57:58], 0.0)
                nc.scalar.activation(
                    C2[0:64, 1:57, 1:57],
                    pl[0:64].rearrange("p (a b) -> p a b", a=56),
                    RELU, bias=t1[0:64, 0:1], scale=s1[0:64, 0:1])
                # upper: rows 0..55 = act(plane rows 0..55), rows 56,57 zero,
                # cols 0,57 zero
                nc.vector.memset(C2[64:128, 56:58, :], 0.0)
                nc.vector.memset(C2[64:128, 0:56, 0:1], 0.0)
                nc.vector.memset(C2[64:128, 0:56, 57:58], 0.0)
                nc.scalar.activation(
                    C2[64:128, 0:56, 1:57],
                    pl[64:128].rearrange("p (a b) -> p a b", a=56),
                    RELU, bias=t1[64:128, 0:1], scale=s1[64:128, 0:1])

            s2prev = None
            for z in range(16):
                B2 = b2p.tile([128, 58, 58], BF16, tag="b2")
                build_b2(B2, z)
                C2 = c2p.tile([128, 58, 58], BF16, tag="c2")
                build_c2(C2, z)
                s2z = c2pool.tile([128, 28, 28], BF16, tag="s2z", bufs=3,
                                  name=f"s2z{z}")
                for p2 in range(4):
                    pst = psum.tile([128, 2, 512], F32, tag="ps", name="ps")
                    for k9 in range(9):
                        kh, kw = k9 // 3, k9 % 3
                        for j in range(2):
                            y0 = 14 * p2 + 7 * j + kh
                            nc.tensor.matmul(pst[:, j, :392], w2a_sb[:, k9, :],
                                             B2[:, y0:y0 + 7, kw:kw + 56],
                                             start=(k9 == 0), stop=False)
                    for kw in range(3):
                        for j in range(2):
                            y0 = 14 * p2 + 7 * j
                            nc.tensor.matmul(pst[:, j, :392], w2bp_sb[:, kw, :],
                                             C2[:, y0:y0 + 7, kw:kw + 56],
                                             start=False, stop=False)
                    for kw in range(3):
                        for j in range(2):
                            y0 = 14 * p2 + 7 * j + 2
                            nc.tensor.matmul(pst[:, j, :392], w2bl_sb[:, kw, :],
                                             C2[:, y0:y0 + 7, kw:kw + 56],
                                             start=False,
                                             stop=(kw == 2 and j == 1))
                    ybft = ybfp.tile([128, 784], BF16, tag="ybf2",
                                     name="ybf")
                    yb2 = ybft[:].rearrange("p (j n) -> p j n", j=2)
                    nc.vector.tensor_copy(yb2, pst[:, :, :392])
                    ti = z * 8 + 2 * p2
                    nc.vector.bn_stats(stats2[:, ti], ybft[:, 0:392])
                    nc.vector.bn_stats(stats2[:, ti + 1], ybft[:, 392:784])
                    v = ybft[:].rearrange("p (a b) -> p a b", a=14)
                    pw = pwp.tile([128, 14, 28], BF16, tag="pw2")
                    nc.vector.tensor_max(pw[:], v[:, :, 0::2], v[:, :, 1::2])
                    nc.vector.tensor_max(s2z[:, 7 * p2:7 * p2 + 7, :],
                                         pw[:, 0::2, :], pw[:, 1::2, :])
                if z % 2 == 1:
                    # incremental z-pool so the boundary only does BN+apply
                    nc.vector.tensor_max(Y2p[:, z // 2], s2prev[:], s2z[:])
                s2prev = s2z
                if z == 13:
                    with nc.named_scope("ar2A"):
                        meA = pack_me(stats2[:, 0:112], 128, "2A")
                        ar2A = cc_allreduce("2A", meA)
        with nc.named_scope("ar2B"):
            meB = pack_me(stats2[:, 112:128], 128, "2B")
            ar2B = cc_allreduce("2B", meB)
            wA, wB = 112.0 / 128.0, 16.0 / 128.0
            me2 = small.tile([128, 2], F32, tag="me2g")
            tmp2 = small.tile([128, 2], F32, tag="tmp2g")
            nc.vector.tensor_scalar_mul(tmp2[:], ar2A[:], wA * inv_n)
            nc.vector.tensor_scalar_mul(me2[:], ar2B[:], wB * inv_n)
            nc.vector.tensor_add(me2[:], me2[:], tmp2[:])
            gt2, bt2 = params["2"]
            s2, t2 = prep_st(me2[:, 0:1], me2[:, 1:2], gt2[:, 0:1], bt2[:, 0:1],
                             128, "2")
        P3in = arena.tile([128, 10, 30, 30], BF16, tag="pin3a")

        def zero_borders1(P, Dp, Hp, Wp):
            nc.vector.memset(P[:, 0], 0.0)
            nc.vector.memset(P[:, Dp - 1], 0.0)
            nc.vector.memset(P[:, 1:Dp - 1, 0, :], 0.0)
            nc.vector.memset(P[:, 1:Dp - 1, Hp - 1, :], 0.0)
            nc.vector.memset(P[:, 1:Dp - 1, 1:Hp - 1, 0:1], 0.0)
            nc.vector.memset(P[:, 1:Dp - 1, 1:Hp - 1, Wp - 1:Wp], 0.0)

        zero_borders1(P3in, 10, 30, 30)
        # split apply: conv3a's first groups need low z planes first
        nc.scalar.activation(P3in[:, 1:4, 1:29, 1:29], Y2p[:, 0:3], RELU,
                             bias=t2[:, 0:1], scale=s2[:, 0:1])
        nc.scalar.activation(P3in[:, 4:9, 1:29, 1:29], Y2p[:, 3:8], RELU,
                             bias=t2[:, 0:1], scale=s2[:, 0:1])
        conv2_pool.__exit__(None, None, None)

        # ---------------- generic conv layers ----------------
        with tc.tile_pool(name="wp", bufs=2) as wp, \
             tc.tile_pool(name="w5p", bufs=6) as w5p, \
             tc.tile_pool(name="stagep", bufs=2) as stagep:
            pins = [P3in]
            for (name, Cin, Cout, D, H, W, R, zpair, pooled) in GEN_LAYERS:
                perchunk_cc = name not in ("5a", "5b")
                Kch, Mch = Cin // 128, Cout // 128
                ntz = D // 2 if zpair else D
                zcnt = 2 if zpair else 1
                ytiles = H // R
                N = zcnt * R * W
                T_chunk = ntz * ytiles
                H2, W2, D2 = H // 2, W // 2, D // 2
                gt, bt = params[name]
                tiles = [(2 * tz if zpair else tz, ty * R)
                         for tz in range(ntz) for ty in range(ytiles)]
                groups = [tiles[i:i + 2] for i in range(0, len(tiles), 2)]
                next_pins = []
                feats = None
                if name == "5b":
                    feats = small.tile([128, 4], F32, tag="feats")
                if name == "3a":
                    stage_tag, stage_bufs = "stage3a", 2
                elif name in ("5a", "5b"):
                    stage_tag, stage_bufs = "stage5", 8
                else:
                    stage_tag, stage_bufs = "stageS", 2
                stages = []
                mebig = None
                if not perchunk_cc:
                    mebig = small.tile([128, Mch, 2], F32, tag="mebig",
                                       name=f"mebig{name}")
                for m in range(Mch):
                    scope = nc.named_scope(f"conv{name}_{m}")
                    scope.__enter__()
                    stats_m = statsp.tile([128, T_chunk, 6], F32, tag="stats_g",
                                          name=f"st{name}{m}", bufs=3)
                    if pooled is False or pooled is None:
                        stage = stagep.tile([128, D, H, W], BF16, tag=stage_tag,
                                            name=f"stage{name}{m}",
                                            bufs=stage_bufs)
                    else:
                        stage = stagep.tile([128, D, H2, W2], BF16,
                                            tag=stage_tag,
                                            name=f"stage{name}{m}",
                                            bufs=stage_bufs)
                    if name not in ("4b", "5a", "5b"):
                        wm = wp.tile([128, Kch, 27, 128], BF16, tag="w",
                                     name=f"w{name}{m}")
                        nc.sync.dma_start(wm[:, :Kch], din[f"w{name}"][m])
                        wref = [wm[:, c] for c in range(Kch)]
                    else:
                        wref = []
                        for c in range(Kch):
                            wc = w5p.tile([128, 27, 128], BF16, tag="w5",
                                          name=f"w{name}{m}c{c}")
                            nc.sync.dma_start(wc[:], din[f"w{name}"][m][:, c])
                            wref.append(wc[:])
                    ti = 0
                    for grp in groups:
                        G = len(grp)
                        pst = psum.tile([128, 2, 512], F32, tag="ps",
                                        name="ps")
                        nmm = Kch * 27
                        i = 0
                        for c in range(Kch):
                            for (kd, kh, kw) in TAPS:
                                for j, (z0, y0) in enumerate(grp):
                                    rhs = pins[c][:, z0 + kd:z0 + kd + zcnt,
                                                  y0 + kh:y0 + kh + R,
                                                  kw:kw + W]
                                    nc.tensor.matmul(
                                        pst[:, j, :N],
                                        wref[c][:, kd * 9 + kh * 3 + kw, :],
                                        rhs, start=(i == 0), stop=(i == nmm - 1))
                                i += 1
                        z0, y0 = grp[0]
                        if pooled is False or pooled is None:
                            # write raw conv outputs into stage
                            dst = stage[:].rearrange("p d h w -> p (d h w)")
                            off = z0 * H * W + y0 * W
                            nc.vector.tensor_copy(
                                dst[:, off:off + G * N].rearrange(
                                    "p (g n) -> p g n", g=G),
                                pst[:, :G, :N])
                            for j in range(G):
                                nc.vector.bn_stats(
                                    stats_m[:, ti + j],
                                    dst[:, off + j * N:off + (j + 1) * N])
                        else:
                            ybft = ybfp.tile([128, 784], BF16,
                                             tag="ybf2", name="ybf")
                            ybf = ybft[:, :G * N]
                            nc.vector.tensor_copy(
                                ybf.rearrange("p (g n) -> p g n", g=G),
                                pst[:, :G, :N])
                            for j in range(G):
                                nc.vector.bn_stats(
                                    stats_m[:, ti + j],
                                    ybft[:, j * N:(j + 1) * N])
                            nz = G * zcnt if zpair else 1
                            nr = R if zpair else G * R
                            v = ybf.rearrange("p (z y x) -> p z y x",
                                              z=nz, y=nr)
                            pw = pwp.tile([128, nz, nr, W2], BF16,
                                          tag="pw3", name="pw")
                            nc.vector.tensor_max(pw[:], v[:, :, :, 0::2],
                                                 v[:, :, :, 1::2])
                            nc.vector.tensor_max(
                                stage[:, z0:z0 + nz,
                                      y0 // 2:y0 // 2 + nr // 2, :],
                                pw[:, :, 0::2, :], pw[:, :, 1::2, :])
                        ti += G
                    scope.__exit__(None, None, None)

                    def apply_chunk(m, stage, s_m, t_m):
                        if name == "5b":
                            # pool5: window (2,2,2) stride 2, pad (0,1,1),
                            # BN+ReLU, global mean (1/16 folded into fcw)
                            pd = small.tile([128, 7, 7], BF16, tag="pd5",
                                            name=f"pd5{m}")
                            nc.vector.tensor_max(pd[:], stage[:, 0],
                                                 stage[:, 1])
                            pw5 = small.tile([128, 7, 4], BF16, tag="pw5",
                                             name=f"pw5{m}")
                            nc.vector.tensor_copy(pw5[:, :, 0:1],
                                                  pd[:, :, 0:1])
                            nc.vector.tensor_max(pw5[:, :, 1:4],
                                                 pd[:, :, 1::2], pd[:, :, 2::2])
                            ph5 = small.tile([128, 4, 4], BF16, tag="ph5",
                                             name=f"ph5{m}")
                            nc.vector.tensor_copy(ph5[:, 0:1, :],
                                                  pw5[:, 0:1, :])
                            nc.vector.tensor_max(ph5[:, 1:4, :],
                                                 pw5[:, 1::2, :],
                                                 pw5[:, 2::2, :])
                            Z = small.tile([128, 16], BF16, tag="z5",
                                           name=f"z5{m}")
                            nc.scalar.activation(
                                Z[:], ph5[:].rearrange("p a b -> p (a b)"),
                                RELU, bias=t_m, scale=s_m)
                            nc.vector.tensor_reduce(feats[:, m:m + 1], Z[:],
                                                    axis=mybir.AxisListType.X,
                                                    op=ADD)
                            return
                        if pooled:
                            src = ypoolp.tile([128, D2, H2, W2], BF16,
                                              tag="ypool", name=f"yp{name}{m}")
                            nc.vector.tensor_max(src[:], stage[:, 0::2],
                                                 stage[:, 1::2])
                            nD, nH, nW = D2, H2, W2
                        else:
                            src = stage
                            nD, nH, nW = D, H, W
                        Pn = arena.tile([128, nD + 2, nH + 2, nW + 2], BF16,
                                        tag=f"pin_{name}",
                                        name=f"pin{name}{m}", bufs=Mch)
                        zero_borders1(Pn, nD + 2, nH + 2, nW + 2)
                        nc.scalar.activation(
                            Pn[:, 1:1 + nD, 1:1 + nH, 1:1 + nW],
                            src[:], RELU, bias=t_m, scale=s_m)
                        next_pins.append(Pn)

                    if perchunk_cc:
                        # per-chunk stats reduce + BN prep + apply, overlapped
                        # with the next chunk's matmuls
                        with nc.named_scope(f"ar{name}_{m}"):
                            me = pack_me(stats_m[:], 128, f"{name}{m}")
                            ar = cc_allreduce(f"{name}{m}", me)
                            mg = small.tile([128, 1], F32, tag="mgc",
                                            name=f"mg{name}{m}")
                            e2 = small.tile([128, 1], F32, tag="e2c",
                                            name=f"e2{name}{m}")
                            nc.vector.tensor_scalar_mul(mg[:], ar[:, 0:1],
                                                        inv_n)
                            nc.vector.tensor_scalar_mul(e2[:], ar[:, 1:2],
                                                        inv_n)
                            s_m, t_m = prep_st(mg, e2, gt[:, m:m + 1],
                                               bt[:, m:m + 1], 128,
                                               f"{name}{m}")
                        apply_chunk(m, stage, s_m[:], t_m[:])
                    else:
                        # defer: aggregate into mebig, one collective at end
                        mv = small.tile([128, 2], F32, tag="mv",
                                        name=f"mv{name}{m}")
                        nc.vector.bn_aggr(mv[:], stats_m[:])
                        sq = small.tile([128, 1], F32, tag="sq",
                                        name=f"sq{name}{m}")
                        nc.vector.tensor_mul(sq[:], mv[:, 0:1], mv[:, 0:1])
                        nc.vector.tensor_copy(mebig[:, m, 0:1], mv[:, 0:1])
                        nc.vector.tensor_add(mebig[:, m, 1:2], mv[:, 1:2],
                                             sq[:])
                        stages.append(stage)

                if not perchunk_cc:
                    with nc.named_scope(f"ar{name}"):
                        ar = cc_allreduce(
                            name, mebig[:].rearrange("p m two -> p (m two)"))
                        arv = ar[:].rearrange("p (m two) -> p m two", two=2)
                        mgw = small.tile([128, Mch], F32, tag="mgw",
                                         name=f"mgw{name}")
                        e2w = small.tile([128, Mch], F32, tag="e2w",
                                         name=f"e2w{name}")
                        nc.vector.tensor_scalar_mul(mgw[:], arv[:, :, 0],
                                                    inv_n)
                        nc.vector.tensor_scalar_mul(e2w[:], arv[:, :, 1],
                                                    inv_n)
                        s_w, t_w = prep_st(mgw, e2w, gt[:, :Mch], bt[:, :Mch],
                                           128, name, F=Mch)
                    for m in range(Mch):
                        apply_chunk(m, stages[m], s_w[:, m:m + 1],
                                    t_w[:, m:m + 1])
                pins = next_pins

            # FC tail
            fcin = small.tile([128, 4], BF16, tag="fcin")
            nc.vector.tensor_copy(fcin[:], feats[:])
            psf = psfc.tile([101, 1], F32, tag="psfc")
            for c in range(4):
                nc.tensor.matmul(psf[:], fcw_sb[:, c, :],
                                 fcin[:, c:c + 1],
                                 start=(c == 0), stop=(c == 3))
            out_sb = small.tile([101, 1], F32, tag="outsb")
            nc.scalar.activation(out_sb[:], psf[:], IDENT,
                                 bias=fcb_sb[:])
            nc.sync.dma_start(logits[:], out_sb[:])


_STATE = {}


def _get_nc(n_cores=N_CORES):
    key = f"nc{n_cores}"
    if key not in _STATE:
        _STATE[key] = build_bass(n_cores)
    return _STATE[key]


def kernel(**inputs):
    nc = _get_nc()
    shared, x1_list = host_prep(inputs)
    in_maps = []
    for i in range(N_CORES):
        m = dict(shared)
        m["x1"] = x1_list[i]
        in_maps.append(m)
    res = run_bass_kernel_spmd(nc, in_maps, core_ids=list(range(N_CORES)))
    out = np.stack([res.results[i]["logits"].reshape(101)
                    for i in range(N_CORES)]).astype(np.float32)
    return out


# revision 50
# speedup vs baseline: 1.0408x; 1.0084x over previous
"""C3D-style circulant-block 3D CNN forward pass on 8 Trainium2 NeuronCores.

Sharding: data-parallel over batch (8 samples -> 8 cores). Training-mode
BatchNorm batch statistics are combined across cores with small per-chunk
f32 AllReduces of (mean, E[x^2]) per channel, issued as soon as each
output-channel chunk (or spatial piece, for single-chunk layers) finishes
so the collective overlaps the remaining matmuls.

Device kernel per core (per sample):
  conv1 via host-side im2col (K=81) -> col-tiled matmuls that compute two
  z-planes concurrently (output channels 0-63 on partitions 0-63 for even
  z, 64-127 for odd z). PSUM->bf16 on ACT, bn_stats on DVE, maxpool
  batched per half-plane on DVE.
  conv2: input channels x kd{0,1} packed into K=128 (B2) + kd=2 handled by
  a row-shift-duplicated plane (C2: parts 64-127 hold the plane shifted by
  one row) so kh taps pair up -> 15 matmuls per tile instead of 18.
  conv3a..5b as shift-and-accumulate implicit GEMM over per-chunk padded
  SBUF inputs; per chunk: stats -> AllReduce -> BN prep -> pool -> fused
  BN+ReLU apply into the next layer's padded chunk, all overlapped with
  the next chunk's matmuls.
  Tail: per-chunk special pool5 -> global mean (folded /16 into FC
  weights) -> FC matmul -> logits.
"""

import numpy as np
import ml_dtypes

import concourse.bass as bass
import concourse.mybir as mybir
import concourse.tile as tile
from concourse import bacc
from concourse.bass_utils import run_bass_kernel_spmd

F32 = mybir.dt.float32
BF16 = mybir.dt.bfloat16
NPBF16 = ml_dtypes.bfloat16
RELU = mybir.ActivationFunctionType.Relu
COPY = mybir.ActivationFunctionType.Copy
SQRT = mybir.ActivationFunctionType.Sqrt
IDENT = mybir.ActivationFunctionType.Identity
ADD = mybir.AluOpType.add
EPS = 1e-5
N_CORES = 8

# name, Cin, Cout, D, H, W, R(rows/tile), zpair, pooled
GEN_LAYERS = [
    ("3a", 128, 256, 8, 28, 28, 14, False, False),
    ("3b", 256, 256, 8, 28, 28, 14, False, True),
    ("4a", 256, 512, 4, 14, 14, 14, True, False),
    ("4b", 512, 512, 4, 14, 14, 14, True, True),
    ("5a", 512, 512, 2, 7, 7, 7, True, False),
    ("5b", 512, 512, 2, 7, 7, 7, True, None),  # None -> raw stage (special pool)
]

TAPS = [(kd, kh, kw) for kd in range(3) for kh in range(3) for kw in range(3)]


def circ_expand_np(c):
    c = np.asarray(c, np.float32)
    P, Q, b = c.shape[0], c.shape[1], c.shape[2]
    r = np.arange(b)
    idx = (r[:, None] - r[None, :]) % b
    w = c[:, :, idx]  # (P, Q, b, b, k, k, k)
    w = np.transpose(w, (0, 2, 1, 3, 4, 5, 6))
    return w.reshape(P * b, Q * b, *c.shape[3:])


def pack_w_generic(wd, Kch, Mch):
    # wd (Co, Ci, 3,3,3) -> [Mch, 128ci, Kch, 27, 128co] bf16
    Co, Ci = wd.shape[0], wd.shape[1]
    wt = wd.transpose(1, 2, 3, 4, 0)  # (Ci, kd,kh,kw, Co)
    wt = wt.reshape(Kch, 128, 27, Mch, 128)
    wt = wt.transpose(3, 1, 0, 2, 4)  # (m, ci, c, t, co)
    return np.ascontiguousarray(wt, dtype=NPBF16)


def host_prep(inputs):
    g = {k: np.asarray(v, np.float32) for k, v in inputs.items()}
    shared = {}
    # conv1
    w1 = g["conv1_w"]  # (64, 3, 3,3,3)
    shared["w1"] = np.ascontiguousarray(
        w1.transpose(1, 2, 3, 4, 0).reshape(81, 64), dtype=NPBF16)
    # conv2 (kd-packed + kd2 row-shift-paired)
    w2 = circ_expand_np(g["c2"])  # (128, 64, 3,3,3)
    w2t = w2.transpose(2, 1, 3, 4, 0)  # (kd, ci, kh, kw, co)
    shared["w2a"] = np.ascontiguousarray(
        w2t[0:2].reshape(128, 9, 128), dtype=NPBF16)
    # kd=2: pair (kh0,kh1) via row-shifted upper partitions; kh2 alone with
    # zeroed upper-partition weights.
    w2b_pair = np.zeros((128, 3, 128), np.float32)
    w2b_pair[0:64] = w2t[2, :, 0]      # (ci, kw, co) for kh=0
    w2b_pair[64:128] = w2t[2, :, 1]    # kh=1 (upper = plane shifted +1 row)
    shared["w2bp"] = np.ascontiguousarray(w2b_pair, dtype=NPBF16)
    w2b_last = np.zeros((128, 3, 128), np.float32)
    w2b_last[0:64] = w2t[2, :, 2]      # kh=2
    shared["w2bl"] = np.ascontiguousarray(w2b_last, dtype=NPBF16)
    # generic layers
    for (name, Cin, Cout, *_rest) in GEN_LAYERS:
        wd = circ_expand_np(g[f"c{name}"])
        shared[f"w{name}"] = pack_w_generic(wd, Cin // 128, Cout // 128)
    # bn params
    def pk(v, parts):
        v = np.asarray(v, np.float32)
        mch = v.size // parts
        return np.ascontiguousarray(v.reshape(mch, parts).T)
    shared["gn1"] = pk(g["g1"], 64)
    shared["bn1"] = pk(g["b1"], 64)
    for name, c in [("2", 128), ("3a", 256), ("3b", 256), ("4a", 512),
                    ("4b", 512), ("5a", 512), ("5b", 512)]:
        shared[f"gn{name}"] = pk(g[f"g{name}"], 128)
        shared[f"bn{name}"] = pk(g[f"b{name}"], 128)
        assert np.all(g[f"g{name}"] >= 0), "pool/BN commute needs g >= 0"
    assert np.all(g["g1"] >= 0)
    # fc (fold /16 global-mean into weights)
    fcw = (g["fc_w"].T / 16.0)  # (512, 101)
    shared["fcw"] = np.ascontiguousarray(
        fcw.reshape(4, 128, 101).transpose(1, 0, 2), dtype=NPBF16)
    shared["fcb"] = np.ascontiguousarray(g["fc_b"].reshape(101, 1))
    # per-core conv1 im2col
    x = g["x"]  # (8, 3, 16, 112, 112)
    x1_list = []
    for i in range(x.shape[0]):
        xp = np.zeros((3, 18, 114, 114), np.float32)
        xp[:, 1:17, 1:113, 1:113] = x[i]
        sw = np.lib.stride_tricks.sliding_window_view(xp, (3, 3, 3), axis=(1, 2, 3))
        b1 = sw.transpose(0, 4, 5, 6, 1, 2, 3).reshape(81, 16, 12544)
        x1_list.append(np.ascontiguousarray(b1, dtype=NPBF16))
    return shared, x1_list


def build_bass(n_cores, fake_cc=False):
    nc = bacc.Bacc("TRN2", target_bir_lowering=False, debug=False,
                   num_devices=n_cores)
    rg = [list(range(n_cores))]

    din = {}
    din["x1"] = nc.dram_tensor("x1", [81, 16, 12544], BF16, kind="ExternalInput")
    din["w1"] = nc.dram_tensor("w1", [81, 64], BF16, kind="ExternalInput")
    din["w2a"] = nc.dram_tensor("w2a", [128, 9, 128], BF16, kind="ExternalInput")
    din["w2bp"] = nc.dram_tensor("w2bp", [128, 3, 128], BF16, kind="ExternalInput")
    din["w2bl"] = nc.dram_tensor("w2bl", [128, 3, 128], BF16, kind="ExternalInput")
    for (name, Cin, Cout, *_r) in GEN_LAYERS:
        din[f"w{name}"] = nc.dram_tensor(
            f"w{name}", [Cout // 128, 128, Cin // 128, 27, 128], BF16,
            kind="ExternalInput")
    din["gn1"] = nc.dram_tensor("gn1", [64, 1], F32, kind="ExternalInput")
    din["bn1"] = nc.dram_tensor("bn1", [64, 1], F32, kind="ExternalInput")
    for name, c in [("2", 128), ("3a", 256), ("3b", 256), ("4a", 512),
                    ("4b", 512), ("5a", 512), ("5b", 512)]:
        mch = c // 128
        din[f"gn{name}"] = nc.dram_tensor(f"gn{name}", [128, mch], F32,
                                          kind="ExternalInput")
        din[f"bn{name}"] = nc.dram_tensor(f"bn{name}", [128, mch], F32,
                                          kind="ExternalInput")
    din["fcw"] = nc.dram_tensor("fcw", [128, 4, 101], BF16, kind="ExternalInput")
    din["fcb"] = nc.dram_tensor("fcb", [101, 1], F32, kind="ExternalInput")
    logits = nc.dram_tensor("logits", [101, 1], F32, kind="ExternalOutput")

    with tile.TileContext(nc) as tc:
        build_graph(tc, din, logits, rg, fake_cc)
    nc.compile()
    return nc


def build_graph(tc, din, logits, rg, fake_cc=False):
    nc = tc.nc
    import contextlib
    ctx = contextlib.ExitStack()
    inv_n = 1.0 / len(rg[0])
    with ctx:
        singles = ctx.enter_context(tc.tile_pool(name="singles", bufs=1))
        small = ctx.enter_context(tc.tile_pool(name="small", bufs=3))
        statsp = ctx.enter_context(tc.tile_pool(name="statsp", bufs=1))
        psum = ctx.enter_context(tc.tile_pool(name="psum", bufs=3, space="PSUM"))
        psfc = ctx.enter_context(tc.tile_pool(name="psfc", bufs=1, space="PSUM"))
        ybfp = ctx.enter_context(tc.tile_pool(name="ybfp", bufs=3))
        pwp = ctx.enter_context(tc.tile_pool(name="pwp", bufs=3))
        ypoolp = ctx.enter_context(tc.tile_pool(name="ypoolp", bufs=2))
        dram = ctx.enter_context(tc.tile_pool(name="dram", bufs=1, space="DRAM"))

        # weights first on the DMA queue
        w1_sb = singles.tile([81, 64], BF16, tag="w1")
        nc.sync.dma_start(w1_sb[:], din["w1"][:])
        w2a_sb = singles.tile([128, 9, 128], BF16, tag="w2a")
        nc.sync.dma_start(w2a_sb[:], din["w2a"][:])
        w2bp_sb = singles.tile([128, 3, 128], BF16, tag="w2bp")
        nc.sync.dma_start(w2bp_sb[:], din["w2bp"][:])
        w2bl_sb = singles.tile([128, 3, 128], BF16, tag="w2bl")
        nc.sync.dma_start(w2bl_sb[:], din["w2bl"][:])
        fcw_sb = singles.tile([128, 4, 101], BF16, tag="fcw")
        nc.sync.dma_start(fcw_sb[:], din["fcw"][:])
        fcb_sb = singles.tile([101, 1], F32, tag="fcb")
        nc.sync.dma_start(fcb_sb[:], din["fcb"][:])

        eps_t = singles.tile([128, 1], F32, tag="eps")
        nc.vector.memset(eps_t[:], EPS)

        # conv1 bn params duplicated to both partition halves
        g1f = singles.tile([128, 1], F32, tag="g1f")
        b1f = singles.tile([128, 1], F32, tag="b1f")
        nc.sync.dma_start(g1f[0:64, :], din["gn1"][:])
        nc.sync.dma_start(g1f[64:128, :], din["gn1"][:])
        nc.sync.dma_start(b1f[0:64, :], din["bn1"][:])
        nc.sync.dma_start(b1f[64:128, :], din["bn1"][:])

        # persistent small params
        params = {}
        for name, parts in [("1", 64), ("2", 128), ("3a", 128), ("3b", 128),
                            ("4a", 128), ("4b", 128), ("5a", 128), ("5b", 128)]:
            mch = din[f"gn{name}"].shape[1]
            gt = singles.tile([parts, mch], F32, tag=f"g{name}")
            bt = singles.tile([parts, mch], F32, tag=f"b{name}")
            nc.sync.dma_start(gt[:], din[f"gn{name}"][:])
            nc.sync.dma_start(bt[:], din[f"bn{name}"][:])
            params[name] = (gt, bt)

        y1_dram = dram.tile([128, 8, 3136], BF16, tag="y1d")

        def cc_allreduce(tag, cc_sb):
            """AllReduce a small [parts, F] f32 SBUF tile; returns SBUF tile."""
            parts, F = cc_sb.shape
            ccin = dram.tile([parts, F], F32, tag=f"ccin{tag}")
            ccout = dram.tile([parts, F], F32, tag=f"ccout{tag}",
                              addr_space="Shared")
            nc.sync.dma_start(ccin[:], cc_sb[:])
            if fake_cc:
                nc.sync.dma_start(ccout[:], ccin[:])
            else:
                nc.gpsimd.collective_compute(
                    "AllReduce", ADD, replica_groups=rg,
                    ins=[ccin.opt()], outs=[ccout.opt()])
            ar = small.tile([parts, F], F32, tag="ar", name=f"ar{tag}")
            nc.sync.dma_start(ar[:], ccout[:])
            return ar

        def pack_me(stats_view, parts, tag):
            """bn_aggr a stats slice -> [parts, 2] (mean, E2) SBUF tile."""
            mv = small.tile([parts, 2], F32, tag="mv", name=f"mv{tag}")
            nc.vector.bn_aggr(mv[:], stats_view)
            sq = small.tile([parts, 1], F32, tag="sq", name=f"sq{tag}")
            nc.vector.tensor_mul(sq[:], mv[:, 0:1], mv[:, 0:1])
            me = small.tile([parts, 2], F32, tag="me", name=f"me{tag}")
            nc.vector.tensor_copy(me[:, 0:1], mv[:, 0:1])
            nc.vector.tensor_add(me[:, 1:2], mv[:, 1:2], sq[:])
            return me

        def prep_st(mg, e2, gt, bt, parts, tag, F=1):
            """From global mean/E2 [parts,F] compute s,t [parts,F]."""
            sq2 = small.tile([parts, F], F32, tag=f"sq2w{F}", name=f"sq2{tag}")
            nc.vector.tensor_mul(sq2[:], mg[:], mg[:])
            varg = small.tile([parts, F], F32, tag=f"vargw{F}",
                              name=f"varg{tag}")
            nc.vector.tensor_sub(varg[:], e2[:], sq2[:])
            sd = small.tile([parts, F], F32, tag=f"sdw{F}", name=f"sd{tag}")
            nc.scalar.activation(sd[:], varg[:], SQRT, bias=eps_t[:parts])
            inv = small.tile([parts, F], F32, tag=f"invw{F}", name=f"inv{tag}")
            nc.vector.reciprocal(inv[:], sd[:])
            s_t = small.tile([parts, F], F32, tag=f"s_tw{F}", name=f"s{tag}")
            nc.vector.tensor_mul(s_t[:], inv[:], gt)
            tmn = small.tile([parts, F], F32, tag=f"tmnw{F}", name=f"tmn{tag}")
            nc.vector.tensor_mul(tmn[:], mg[:], s_t[:])
            t_t = small.tile([parts, F], F32, tag=f"t_tw{F}", name=f"t{tag}")
            nc.vector.tensor_sub(t_t[:], bt, tmn[:])
            return s_t, t_t

        # ---------------- conv1 ----------------
        # stats windows: 8 zp x 2 half x 7 t2 = 112 entries of [128, 2, 448]
        stats1 = statsp.tile([128, 224, 6], F32, tag="stats1")
        # stats split: A (zp0-5, absorbs initial core skew while zp6-7
        # compute), B (zp6-7, exposed ~wire latency only)
        PCS = [(0, 168, "1A", 5), (168, 224, "1B", None)]
        ars1 = {}
        with tc.tile_pool(name="x1p", bufs=3) as x1p, \
             tc.tile_pool(name="zplp", bufs=2) as zplp, \
             nc.named_scope("conv1"):
            for zp in range(8):
                zplane = zplp.tile([128, 56, 56], BF16, tag="zpl")
                for half in range(2):
                    slab = x1p.tile([81, 2, 6272], BF16, tag="slab")
                    for zg in range(2):
                        src = din["x1"][:, 2 * zp + zg,
                                        half * 6272:(half + 1) * 6272]
                        if zp == 0 and half == 0:
                            # finer first chunks so matmuls start earlier
                            for q in range(4):
                                nc.sync.dma_start(
                                    slab[:, zg, q * 1568:(q + 1) * 1568],
                                    src[:, q * 1568:(q + 1) * 1568])
                        else:
                            nc.sync.dma_start(slab[:, zg], src)
                    ybfh = x1p.tile([128, 6272], BF16, tag="ybf",
                                    name="ybfh", bufs=2)
                    for t2 in range(7):
                        pst = psum.tile([128, 2, 512], F32, tag="ps", name="ps")
                        for j in range(2):
                            t = 2 * t2 + j
                            for zg in range(2):
                                nc.tensor.matmul(
                                    pst[zg * 64:(zg + 1) * 64, j, :448],
                                    w1_sb[:],
                                    slab[:, zg, t * 448:(t + 1) * 448],
                                    start=True, stop=True,
                                    tile_position=(0, 64 * zg))
                        yb2 = ybfh[:, t2 * 896:(t2 + 1) * 896].rearrange(
                            "p (j n) -> p j n", j=2)
                        nc.scalar.activation(yb2, pst[:, :, :448], COPY)
                        ti = (zp * 2 + half) * 14 + 2 * t2
                        nc.vector.bn_stats(stats1[:, ti],
                                           ybfh[:, t2 * 896:t2 * 896 + 448])
                        nc.vector.bn_stats(stats1[:, ti + 1],
                                           ybfh[:, t2 * 896 + 448:(t2 + 1) * 896])
                    # batched pools: one x-pool + one y-pool per half
                    vh = ybfh[:].rearrange("p (a b) -> p a b", a=56)
                    pwh = x1p.tile([128, 56, 56], BF16, tag="pw", bufs=2)
                    nc.vector.tensor_max(pwh[:], vh[:, :, 0::2], vh[:, :, 1::2])
                    nc.vector.tensor_max(zplane[:, half * 28:half * 28 + 28, :],
                                         pwh[:, 0::2, :], pwh[:, 1::2, :])
                nc.sync.dma_start(y1_dram[:, zp, :],
                                  zplane[:].rearrange("p a b -> p (a b)"))
                for (a, b, nm, at_zp) in PCS:
                    if at_zp == zp:
                        with nc.named_scope(f"ar{nm}"):
                            me = pack_me(stats1[:, a:b], 128, nm)
                            ars1[nm] = cc_allreduce(nm, me)
        with nc.named_scope("ar1C"):
            a, b, nm, _ = PCS[-1]
            me = pack_me(stats1[:, a:b], 128, nm)
            ars1[nm] = cc_allreduce(nm, me)
            # combine pieces (weighted by window counts)
            mh = small.tile([128, 2], F32, tag="mh")  # (mean, E2) per half
            tmpA = small.tile([128, 2], F32, tag="tmpA")
            nc.vector.tensor_scalar_mul(mh[:], ars1["1A"][:],
                                        (168.0 / 224.0) * inv_n)
            nc.vector.tensor_scalar_mul(tmpA[:], ars1["1B"][:],
                                        (56.0 / 224.0) * inv_n)
            nc.vector.tensor_add(mh[:], mh[:], tmpA[:])
            # cross-half combine via partition swap (parts 64-127 = odd z)
            mhs = small.tile([128, 2], F32, tag="mhs")
            nc.sync.dma_start(mhs[0:64, :], mh[64:128, :])
            nc.sync.dma_start(mhs[64:128, :], mh[0:64, :])
            me1 = small.tile([128, 2], F32, tag="me1g")
            nc.vector.tensor_add(me1[:], mh[:], mhs[:])
            nc.vector.tensor_scalar_mul(me1[:], me1[:], 0.5)
            s1, t1 = prep_st(me1[:, 0:1], me1[:, 1:2], g1f[:, 0:1], b1f[:, 0:1],
                             128, "1")

        # ---------------- conv2 ----------------
        # stats: 16 z x 4 p2 x 2 j = 128 entries of [128, 392]
        stats2 = statsp.tile([128, 128, 6], F32, tag="stats2")
        arena = ctx.enter_context(tc.tile_pool(name="arena", bufs=1))
        conv2_pool = tc.tile_pool(name="c2pool", bufs=1)
        c2pool = conv2_pool.__enter__()
        Y2p = c2pool.tile([128, 8, 28, 28], BF16, tag="y2pool")
        with tc.tile_pool(name="plp", bufs=2) as plp, \
             tc.tile_pool(name="b2p", bufs=2) as b2p, \
             tc.tile_pool(name="c2p", bufs=2) as c2p, \
             nc.named_scope("conv2"):

            def load_plane(dst, pidx):
                """DMA y1 plane (padded idx) into a [64,...] partition range."""
                z = pidx - 1
                nc.sync.dma_start(dst, y1_dram[64 * (z % 2):64 * (z % 2) + 64,
                                               z // 2, :])

            def build_b2(B2, z):
                """B2: parts 0-63 = padded plane z, 64-127 = padded plane z+1."""
                p0, p1 = z, z + 1  # padded plane indices
                pl = plp.tile([128, 3136], BF16, tag="pl")
                halves = []
                for hi, pidx in enumerate((p0, p1)):
                    sl = slice(64 * hi, 64 * hi + 64)
                    if pidx == 0 or pidx == 17:
                        nc.vector.memset(B2[sl], 0.0)
                    else:
                        load_plane(pl[sl], pidx)
                        halves.append(sl)
                        nc.vector.memset(B2[sl, 0, :], 0.0)
                        nc.vector.memset(B2[sl, 57, :], 0.0)
                        nc.vector.memset(B2[sl, 1:57, 0:1], 0.0)
                        nc.vector.memset(B2[sl, 1:57, 57:58], 0.0)
                if len(halves) == 2:
                    nc.scalar.activation(
                        B2[:, 1:57, 1:57],
                        pl[:].rearrange("p (a b) -> p a b", a=56),
                        RELU, bias=t1[:, 0:1], scale=s1[:, 0:1])
                elif len(halves) == 1:
                    sl = halves[0]
                    nc.scalar.activation(
                        B2[sl, 1:57, 1:57],
                        pl[sl].rearrange("p (a b) -> p a b", a=56),
                        RELU, bias=t1[sl, 0:1], scale=s1[sl, 0:1])

            def build_c2(C2, z):
                """C2: parts 0-63 = padded plane z+2; parts 64-127 = same
                plane shifted up one row (row y = padded row y+1)."""
                pidx = z + 2
                if pidx == 17:
                    nc.vector.memset(C2[:], 0.0)
                    return
                pl = plp.tile([128, 3136], BF16, tag="pl")
                load_plane(pl[0:64], pidx)
                load_plane(pl[64:128], pidx)
                # lower: normal padded plane
                nc.vector.memset(C2[0:64, 0, :], 0.0)
                nc.vector.memset(C2[0:64, 57, :], 0.0)
                nc.vector.memset(C2[0:64, 1:57, 0:1], 0.0)
                nc.vector.memset(C2[0:64, 1:57, 57:58], 0.0)
                nc.scalar.activation(
                    C2[0:64, 1:57, 1:57],
                    pl[0:64].rearrange("p (a b) -> p a b", a=56),
                    RELU, bias=t1[0:64, 0:1], scale=s1[0:64, 0:1])
                # upper: rows 0..55 = act(plane rows 0..55), rows 56,57 zero,
                # cols 0,57 zero
                nc.vector.memset(C2[64:128, 56:58, :], 0.0)
                nc.vector.memset(C2[64:128, 0:56, 0:1], 0.0)
                nc.vector.memset(C2[64:128, 0:56, 57:58], 0.0)
                nc.scalar.activation(
                    C2[64:128, 0:56, 1:57],
                    pl[64:128].rearrange("p (a b) -> p a b", a=56),
                    RELU, bias=t1[64:128, 0:1], scale=s1[64:128, 0:1])

            s2prev = None
            for z in range(16):
                B2 = b2p.tile([128, 58, 58], BF16, tag="b2")
                build_b2(B2, z)
                C2 = c2p.tile([128, 58, 58], BF16, tag="c2")
                build_c2(C2, z)
                s2z = c2pool.tile([128, 28, 28], BF16, tag="s2z", bufs=3,
                                  name=f"s2z{z}")
                for p2 in range(4):
                    pst = psum.tile([128, 2, 512], F32, tag="ps", name="ps")
                    for k9 in range(9):
                        kh, kw = k9 // 3, k9 % 3
                        for j in range(2):
                            y0 = 14 * p2 + 7 * j + kh
                            nc.tensor.matmul(pst[:, j, :392], w2a_sb[:, k9, :],
                                             B2[:, y0:y0 + 7, kw:kw + 56],
                                             start=(k9 == 0), stop=False)
                    for kw in range(3):
                        for j in range(2):
                            y0 = 14 * p2 + 7 * j
                            nc.tensor.matmul(pst[:, j, :392], w2bp_sb[:, kw, :],
                                             C2[:, y0:y0 + 7, kw:kw + 56],
                                             start=False, stop=False)
                    for kw in range(3):
                        for j in range(2):
                            y0 = 14 * p2 + 7 * j + 2
                            nc.tensor.matmul(pst[:, j, :392], w2bl_sb[:, kw, :],
                                             C2[:, y0:y0 + 7, kw:kw + 56],
                                             start=False,
                                             stop=(kw == 2 and j == 1))
                    ybft = ybfp.tile([128, 784], BF16, tag="ybf2",
                                     name="ybf")
                    yb2 = ybft[:].rearrange("p (j n) -> p j n", j=2)
                    nc.vector.tensor_copy(yb2, pst[:, :, :392])
                    ti = z * 8 + 2 * p2
                    nc.vector.bn_stats(stats2[:, ti], ybft[:, 0:392])
                    nc.vector.bn_stats(stats2[:, ti + 1], ybft[:, 392:784])
                    v = ybft[:].rearrange("p (a b) -> p a b", a=14)
                    pw = pwp.tile([128, 14, 28], BF16, tag="pw2")
                    nc.vector.tensor_max(pw[:], v[:, :, 0::2], v[:, :, 1::2])
                    nc.vector.tensor_max(s2z[:, 7 * p2:7 * p2 + 7, :],
                                         pw[:, 0::2, :], pw[:, 1::2, :])
                if z % 2 == 1:
                    # incremental z-pool so the boundary only does BN+apply
                    nc.vector.tensor_max(Y2p[:, z // 2], s2prev[:], s2z[:])
                s2prev = s2z
                if z == 11:
                    with nc.named_scope("ar2A"):
                        meA = pack_me(stats2[:, 0:96], 128, "2A")
                        ar2A = cc_allreduce("2A", meA)
                if z == 14:
                    with nc.named_scope("ar2B"):
                        meB = pack_me(stats2[:, 96:120], 128, "2B")
                        ar2B = cc_allreduce("2B", meB)
        with nc.named_scope("ar2C"):
            meC = pack_me(stats2[:, 120:128], 128, "2C")
            ar2C = cc_allreduce("2C", meC)
            me2 = small.tile([128, 2], F32, tag="me2g")
            tmp2 = small.tile([128, 2], F32, tag="tmp2g")
            nc.vector.tensor_scalar_mul(me2[:], ar2A[:],
                                        (96.0 / 128.0) * inv_n)
            nc.vector.tensor_scalar_mul(tmp2[:], ar2B[:],
                                        (24.0 / 128.0) * inv_n)
            nc.vector.tensor_add(me2[:], me2[:], tmp2[:])
            nc.vector.tensor_scalar_mul(tmp2[:], ar2C[:],
                                        (8.0 / 128.0) * inv_n)
            nc.vector.tensor_add(me2[:], me2[:], tmp2[:])
            gt2, bt2 = params["2"]
            s2, t2 = prep_st(me2[:, 0:1], me2[:, 1:2], gt2[:, 0:1], bt2[:, 0:1],
                             128, "2")
        P3in = arena.tile([128, 10, 30, 30], BF16, tag="pin3a")

        def zero_borders1(P, Dp, Hp, Wp):
            nc.vector.memset(P[:, 0], 0.0)
            nc.vector.memset(P[:, Dp - 1], 0.0)
            nc.vector.memset(P[:, 1:Dp - 1, 0, :], 0.0)
            nc.vector.memset(P[:, 1:Dp - 1, Hp - 1, :], 0.0)
            nc.vector.memset(P[:, 1:Dp - 1, 1:Hp - 1, 0:1], 0.0)
            nc.vector.memset(P[:, 1:Dp - 1, 1:Hp - 1, Wp - 1:Wp], 0.0)

        zero_borders1(P3in, 10, 30, 30)
        # split apply: conv3a's first groups need low z planes first
        nc.scalar.activation(P3in[:, 1:4, 1:29, 1:29], Y2p[:, 0:3], RELU,
                             bias=t2[:, 0:1], scale=s2[:, 0:1])
        nc.scalar.activation(P3in[:, 4:9, 1:29, 1:29], Y2p[:, 3:8], RELU,
                             bias=t2[:, 0:1], scale=s2[:, 0:1])
        conv2_pool.__exit__(None, None, None)

        # ---------------- generic conv layers ----------------
        with tc.tile_pool(name="wp", bufs=2) as wp, \
             tc.tile_pool(name="w5p", bufs=6) as w5p, \
             tc.tile_pool(name="stagep", bufs=2) as stagep:
            pins = [P3in]
            for (name, Cin, Cout, D, H, W, R, zpair, pooled) in GEN_LAYERS:
                perchunk_cc = name not in ("5a", "5b")
                Kch, Mch = Cin // 128, Cout // 128
                ntz = D // 2 if zpair else D
                zcnt = 2 if zpair else 1
                ytiles = H // R
                N = zcnt * R * W
                T_chunk = ntz * ytiles
                H2, W2, D2 = H // 2, W // 2, D // 2
                gt, bt = params[name]
                tiles = [(2 * tz if zpair else tz, ty * R)
                         for tz in range(ntz) for ty in range(ytiles)]
                groups = [tiles[i:i + 2] for i in range(0, len(tiles), 2)]
                next_pins = []
                feats = None
                if name == "5b":
                    feats = small.tile([128, 4], F32, tag="feats")
                if name == "3a":
                    stage_tag, stage_bufs = "stage3a", 2
                elif name in ("5a", "5b"):
                    stage_tag, stage_bufs = "stage5", 8
                else:
                    stage_tag, stage_bufs = "stageS", 2
                stages = []
                mebig = None
                if not perchunk_cc:
                    mebig = small.tile([128, Mch, 2], F32, tag="mebig",
                                       name=f"mebig{name}")
                for m in range(Mch):
                    scope = nc.named_scope(f"conv{name}_{m}")
                    scope.__enter__()
                    stats_m = statsp.tile([128, T_chunk, 6], F32, tag="stats_g",
                                          name=f"st{name}{m}", bufs=3)
                    if pooled is False or pooled is None:
                        stage = stagep.tile([128, D, H, W], BF16, tag=stage_tag,
                                            name=f"stage{name}{m}",
                                            bufs=stage_bufs)
                    else:
                        stage = stagep.tile([128, D, H2, W2], BF16,
                                            tag=stage_tag,
                                            name=f"stage{name}{m}",
                                            bufs=stage_bufs)
                    if name not in ("4b", "5a", "5b"):
                        wm = wp.tile([128, Kch, 27, 128], BF16, tag="w",
                                     name=f"w{name}{m}")
                        nc.sync.dma_start(wm[:, :Kch], din[f"w{name}"][m])
                        wref = [wm[:, c] for c in range(Kch)]
                    else:
                        wref = []
                        for c in range(Kch):
                            wc = w5p.tile([128, 27, 128], BF16, tag="w5",
                                          name=f"w{name}{m}c{c}")
                            nc.sync.dma_start(wc[:], din[f"w{name}"][m][:, c])
                            wref.append(wc[:])
                    ti = 0
                    for grp in groups:
                        G = len(grp)
                        pst = psum.tile([128, 2, 512], F32, tag="ps",
                                        name="ps")
                        nmm = Kch * 27
                        i = 0
                        for c in range(Kch):
                            for (kd, kh, kw) in TAPS:
                                for j, (z0, y0) in enumerate(grp):
                                    rhs = pins[c][:, z0 + kd:z0 + kd + zcnt,
                                                  y0 + kh:y0 + kh + R,
                                                  kw:kw + W]
                                    nc.tensor.matmul(
                                        pst[:, j, :N],
                                        wref[c][:, kd * 9 + kh * 3 + kw, :],
                                        rhs, start=(i == 0), stop=(i == nmm - 1))
                                i += 1
                        z0, y0 = grp[0]
                        if pooled is False or pooled is None:
                            # write raw conv outputs into stage
                            dst = stage[:].rearrange("p d h w -> p (d h w)")
                            off = z0 * H * W + y0 * W
                            nc.vector.tensor_copy(
                                dst[:, off:off + G * N].rearrange(
                                    "p (g n) -> p g n", g=G),
                                pst[:, :G, :N])
                            for j in range(G):
                                nc.vector.bn_stats(
                                    stats_m[:, ti + j],
                                    dst[:, off + j * N:off + (j + 1) * N])
                        else:
                            ybft = ybfp.tile([128, 784], BF16,
                                             tag="ybf2", name="ybf")
                            ybf = ybft[:, :G * N]
                            nc.vector.tensor_copy(
                                ybf.rearrange("p (g n) -> p g n", g=G),
                                pst[:, :G, :N])
                            for j in range(G):
                                nc.vector.bn_stats(
                                    stats_m[:, ti + j],
                                    ybft[:, j * N:(j + 1) * N])
                            nz = G * zcnt if zpair else 1
                            nr = R if zpair else G * R
                            v = ybf.rearrange("p (z y x) -> p z y x",
                                              z=nz, y=nr)
                            pw = pwp.tile([128, nz, nr, W2], BF16,
                                          tag="pw3", name="pw")
                            nc.vector.tensor_max(pw[:], v[:, :, :, 0::2],
                                                 v[:, :, :, 1::2])
                            nc.vector.tensor_max(
                                stage[:, z0:z0 + nz,
                                      y0 // 2:y0 // 2 + nr // 2, :],
                                pw[:, :, 0::2, :], pw[:, :, 1::2, :])
                        ti += G
                    scope.__exit__(None, None, None)

                    def apply_chunk(m, stage, s_m, t_m):
                        if name == "5b":
                            # pool5: window (2,2,2) stride 2, pad (0,1,1),
                            # BN+ReLU, global mean (1/16 folded into fcw)
                            pd = small.tile([128, 7, 7], BF16, tag="pd5",
                                            name=f"pd5{m}")
                            nc.vector.tensor_max(pd[:], stage[:, 0],
                                                 stage[:, 1])
                            pw5 = small.tile([128, 7, 4], BF16, tag="pw5",
                                             name=f"pw5{m}")
                            nc.vector.tensor_copy(pw5[:, :, 0:1],
                                                  pd[:, :, 0:1])
                            nc.vector.tensor_max(pw5[:, :, 1:4],
                                                 pd[:, :, 1::2], pd[:, :, 2::2])
                            ph5 = small.tile([128, 4, 4], BF16, tag="ph5",
                                             name=f"ph5{m}")
                            nc.vector.tensor_copy(ph5[:, 0:1, :],
                                                  pw5[:, 0:1, :])
                            nc.vector.tensor_max(ph5[:, 1:4, :],
                                                 pw5[:, 1::2, :],
                                                 pw5[:, 2::2, :])
                            Z = small.tile([128, 16], BF16, tag="z5",
                                           name=f"z5{m}")
                            nc.scalar.activation(
                                Z[:], ph5[:].rearrange("p a b -> p (a b)"),
                                RELU, bias=t_m, scale=s_m)
                            nc.vector.tensor_reduce(feats[:, m:m + 1], Z[:],
                                                    axis=mybir.AxisListType.X,
                                                    op=ADD)
                            return
                        if pooled:
                            src = ypoolp.tile([128, D2, H2, W2], BF16,
                                              tag="ypool", name=f"yp{name}{m}")
                            nc.vector.tensor_max(src[:], stage[:, 0::2],
                                                 stage[:, 1::2])
                            nD, nH, nW = D2, H2, W2
                        else:
                            src = stage
                            nD, nH, nW = D, H, W
                        Pn = arena.tile([128, nD + 2, nH + 2, nW + 2], BF16,
                                        tag=f"pin_{name}",
                                        name=f"pin{name}{m}", bufs=Mch)
                        zero_borders1(Pn, nD + 2, nH + 2, nW + 2)
                        nc.scalar.activation(
                            Pn[:, 1:1 + nD, 1:1 + nH, 1:1 + nW],
                            src[:], RELU, bias=t_m, scale=s_m)
                        next_pins.append(Pn)

                    if perchunk_cc:
                        # per-chunk stats reduce + BN prep + apply, overlapped
                        # with the next chunk's matmuls
                        with nc.named_scope(f"ar{name}_{m}"):
                            me = pack_me(stats_m[:], 128, f"{name}{m}")
                            ar = cc_allreduce(f"{name}{m}", me)
                            mg = small.tile([128, 1], F32, tag="mgc",
                                            name=f"mg{name}{m}")
                            e2 = small.tile([128, 1], F32, tag="e2c",
                                            name=f"e2{name}{m}")
                            nc.vector.tensor_scalar_mul(mg[:], ar[:, 0:1],
                                                        inv_n)
                            nc.vector.tensor_scalar_mul(e2[:], ar[:, 1:2],
                                                        inv_n)
                            s_m, t_m = prep_st(mg, e2, gt[:, m:m + 1],
                                               bt[:, m:m + 1], 128,
                                               f"{name}{m}")
                        apply_chunk(m, stage, s_m[:], t_m[:])
                    else:
                        # defer: aggregate into mebig, one collective at end
                        mv = small.tile([128, 2], F32, tag="mv",
                                        name=f"mv{name}{m}")
                        nc.vector.bn_aggr(mv[:], stats_m[:])
                        sq = small.tile([128, 1], F32, tag="sq",
                                        name=f"sq{name}{m}")
                        nc.vector.tensor_mul(sq[:], mv[:, 0:1], mv[:, 0:1])
                        nc.vector.tensor_copy(mebig[:, m, 0:1], mv[:, 0:1])
                        nc.vector.tensor_add(mebig[:, m, 1:2], mv[:, 1:2],
                                             sq[:])
                        stages.append(stage)

                if not perchunk_cc:
                    with nc.named_scope(f"ar{name}"):
                        ar = cc_allreduce(
                            name, mebig[:].rearrange("p m two -> p (m two)"))
                        arv = ar[:].rearrange("p (m two) -> p m two", two=2)
                        mgw = small.tile([128, Mch], F32, tag="mgw",
                                         name=f"mgw{name}")
                        e2w = small.tile([128, Mch], F32, tag="e2w",
                                         name=f"e2w{name}")
                        nc.vector.tensor_scalar_mul(mgw[:], arv[:, :, 0],
                                                    inv_n)
                        nc.vector.tensor_scalar_mul(e2w[:], arv[:, :, 1],
                                                    inv_n)
                        s_w, t_w = prep_st(mgw, e2w, gt[:, :Mch], bt[:, :Mch],
                                           128, name, F=Mch)
                    for m in range(Mch):
                        apply_chunk(m, stages[m], s_w[:, m:m + 1],
                                    t_w[:, m:m + 1])
                pins = next_pins

            # FC tail
            fcin = small.tile([128, 4], BF16, tag="fcin")
            nc.vector.tensor_copy(fcin[:], feats[:])
            psf = psfc.tile([101, 1], F32, tag="psfc")
            for c in range(4):
                nc.tensor.matmul(psf[:], fcw_sb[:, c, :],
                                 fcin[:, c:c + 1],
                                 start=(c == 0), stop=(c == 3))
            out_sb = small.tile([101, 1], F32, tag="outsb")
            nc.scalar.activation(out_sb[:], psf[:], IDENT,
                                 bias=fcb_sb[:])
            nc.sync.dma_start(logits[:], out_sb[:])


_STATE = {}


def _get_nc(n_cores=N_CORES):
    key = f"nc{n_cores}"
    if key not in _STATE:
        _STATE[key] = build_bass(n_cores)
    return _STATE[key]


def kernel(**inputs):
    nc = _get_nc()
    shared, x1_list = host_prep(inputs)
    in_maps = []
    for i in range(N_CORES):
        m = dict(shared)
        m["x1"] = x1_list[i]
        in_maps.append(m)
    res = run_bass_kernel_spmd(nc, in_maps, core_ids=list(range(N_CORES)))
    out = np.stack([res.results[i]["logits"].reshape(101)
                    for i in range(N_CORES)]).astype(np.float32)
    return out
